# revision 1
# baseline (speedup 1.0000x reference)
"""Trainium2 Bass kernel for nn_EnergyBalanceChecker (segment_reduce), v4.

Problem (hardcoded): B=4, N=512, T=24, G=32, TOL=0.05, EPS=1e-6.

  M = onehot(lv_group_ids) * valid_lv_mask                     # [G, N]
  gc  = einsum('gn,bnt->bgt', M, consumption)
  gg  = einsum('gn,bnt->bgt', M, generation)
  net = einsum('gn,bnt->bgt', M, S.sum(axis=2) - S.sum(axis=1))
  pen = relu(|gc-gg+net| / (gc+gg+eps) - TOL);  out = pen.sum()*w/n_unique

Sharding: 8 cores = 4 batches x 2 halves of the (row) N axis.

Dataflow:
  * S streams in fp8e4 (SWDGE cast in flight): DMA cost is charged on
    destination bytes, so this halves the 17.5us fp16 stream floor.
  * The 128 SBUF partitions carry (na in 8 n-rows) x (msub in 16 m-blocks),
    so each matmul contracts 16 m-positions along with n. 32 blocks of 8
    n-rows cover the shard; lhsT = [M-projection rows | 16 msub indicator
    rows] (indicator rows preserve per-m column sums).
  * Blocks pair into fp8 DoubleRow matmuls (2 k-tiles/pass, 0.5 cyc/col)
    accumulating into per-half PSUM regions, column-split into [*,512] +
    [*,256] tiles so the two drain copies (ACT / DVE) run concurrently.
  * Drain: copy region to f16, then PE folds the q axis -- identity-matmul
    the projection rows (+row term) and (-M-slice)-matmul the colsum rows
    (-col term) -- accumulating net = row - col directly into one PSUM tile.
  * Host does only the [3, G, T]-level nonlinear tail.
"""

import sys

import numpy as np

try:
    import concourse  # noqa: F401
except ImportError:
    sys.path.insert(0, "/opt/trn_rl_repo")

import ml_dtypes

import concourse.tile as tile
from concourse import bacc, mybir
from concourse.bass_utils import run_bass_kernel_spmd

B, N, T, G = 4, 512, 24, 32
TOL, EPS = 0.05, 1e-6
P = 128                 # SBUF partitions
NLOC = N // 2           # rows per core (n-half)
A = 8                   # n-rows per block (partition sub-dim)
C = 16                  # m-blocks on partitions (partition sub-dim)
Q = N // C              # m-columns per msub block (free dim)
NBLK = NLOC // A        # 32 blocks of 8 n-rows
PAIRS = NBLK // 2       # DoubleRow pairs
F = Q * T               # free elements per block / PSUM region columns
GC = G + C              # lhsT columns: G projection + C indicator rows
DMA_BLOCKS = ((0, 5), (5, 11), (11, 17), (17, 23), (23, 28),
              (28, 30), (30, 32))   # stream DMA block ranges
CHUNKS = ((0, 21 * T), (21 * T, F - 21 * T))   # q-aligned matmul chunks
QSPLIT = 21             # chunk boundary in units of q
OROW = 128              # padded f32 row per group in the scatter-add out

_F32 = mybir.dt.float32
_F16 = mybir.dt.float16
_F8 = mybir.dt.float8e4
_U8 = mybir.dt.uint8
_I16 = mybir.dt.int16


def _build_program():
    nc = bacc.Bacc("TRN2", target_bir_lowering=False, debug=False,
                   enable_asserts=False, num_devices=8)
    s = nc.dram_tensor("s", [NLOC, N, T], _F32, kind="ExternalInput").ap()
    sm_in = nc.dram_tensor("sm_in", [P, 2 * (G + 2 * T)], _F32,
                           kind="ExternalInput").ap()
    lhs8 = nc.dram_tensor("lhs8", [P, NBLK, GC], _U8, kind="ExternalInput").ap()
    msl = nc.dram_tensor("msl", [C, Q, G], _F16, kind="ExternalInput").ap()
    out = nc.dram_tensor("out", [3, G, T], _F32, kind="ExternalOutput").ap()

    with tile.TileContext(nc) as tc, nc.allow_low_precision("f16 rowsum partials, |rs|<300 vs 2e-2 tol"):
        with (
            tc.tile_pool(name="sb", bufs=1) as sb,
            tc.tile_pool(name="ps", bufs=1, space="PSUM") as ps,
        ):
            # --- small inputs on the SP HWDGE ring ---
            ltile = sb.tile([P, NBLK, GC], _U8, tag="ltile")
            nc.sync.dma_start(out=ltile, in_=lhs8)
            # msl lands on partitions G..G+C so the col-matmul lhsT shares the
            # base partition of the colsum rows in pcopy.
            msl_sb = sb.tile([GC, Q, G], _F16, tag="msl")
            nc.sync.dma_start(out=msl_sb[G:GC], in_=msl)
            lhsT = ltile.bitcast(_F8)

            stile = sb.tile([P, NBLK, F], _F8, tag="stile")
            pcopy = sb.tile([GC, 2, F], _F16, tag="pcopy")
            small = sb.tile([P, 2, G + 2 * T], _F32, tag="small")
            rs = sb.tile([G, 2, T], _F16, tag="rs")
            rsp = sb.tile([G, 2, T], _F16, tag="rsp")
            rsum = sb.tile([G, T], _F16, tag="rsum")
            osb = sb.tile([G, 3, T], _F32, tag="osb")

            # Per-half regions, column-split so the two drain copies hit
            # different PSUM tiles and run concurrently (the dep tracker
            # serializes readers of a single PSUM tile).
            reg = [(ps.tile([GC, CHUNKS[0][1]], _F32, name=f"reg{h}a"),
                    ps.tile([GC, CHUNKS[1][1]], _F32, name=f"reg{h}b"))
                   for h in range(2)]
            bt = ps.tile([G, T], _F32, tag="bt")
            pg = ps.tile([G, 2 * T], _F32, tag="pg")

            # --- S stream: fp8 cast DMAs on the SWDGE ring ---
            # partition p = na*C + msub; block blk: n = blk*A + na;
            # free = (mq, t) with m = msub*Q + mq.
            s_r = s.rearrange("(blk a) (c q) t -> (a c) blk (q t)", a=A, c=C)
            for b0, b1 in DMA_BLOCKS:
                nc.gpsimd.dma_start(
                    out=stile[:, b0:b1, :].rearrange("p b f -> p (b f)"),
                    in_=s_r[:, b0:b1, :])
            # sm_in rides the SWDGE ring after the stream so its transfer
            # lands past the last S byte instead of injecting mid-stream.
            nc.gpsimd.dma_start(
                out=small.rearrange("p nb f -> p (nb f)"), in_=sm_in)

            # --- main pass: DoubleRow matmuls, two half-regions ---
            # Drain: chunks are q-aligned (21q | 11q), so DVE reduces its
            # chunk's projection rows straight from PSUM (no copy, no
            # cross-engine wait) and copies only its colsum rows, while ACT
            # copies the big chunk in full; DVE then reduces the big chunk
            # from the f16 copy. PE's -M col-matmuls fold the colsum rows
            # into bt; the q-slices split cleanly at the chunk boundary.
            def drain(h):
                # DVE's small-chunk reduce reads PSUM directly and starts at
                # region-stop with no cross-engine wait; ACT copies the big
                # chunk in full plus the small chunk's colsum rows.
                nc.vector.reduce_sum(
                    out=rsp[:, 1],
                    in_=reg[h][1][0:G].rearrange("p (q t) -> p t q", t=T),
                    axis=mybir.AxisListType.X)
                nc.scalar.copy(out=pcopy[:, h, 0:CHUNKS[0][1]], in_=reg[h][0])
                nc.scalar.copy(out=pcopy[G:GC, h, CHUNKS[0][1]:],
                               in_=reg[h][1][G:GC])
                if h == 1:
                    # Fold region A's total in early, while ACT still copies.
                    nc.vector.tensor_add(rs[:, 1], rsp[:, 1], rs[:, 0])
                nc.vector.reduce_sum(
                    out=rsp[:, 0],
                    in_=pcopy[0:G, h, 0:CHUNKS[0][1]].rearrange(
                        "p (q t) -> p t q", t=T),
                    axis=mybir.AxisListType.X)
                if h == 0:
                    nc.vector.tensor_add(rs[:, 0], rsp[:, 0], rsp[:, 1])
                else:
                    nc.vector.tensor_add(rsum, rsp[:, 0], rs[:, 1])
                for q in range(Q):
                    nc.tensor.matmul(
                        bt, msl_sb[G:GC, q],
                        pcopy[G:GC, h, q * T:(q + 1) * T],
                        start=(h == 0 and q == 0),
                        stop=(h == 1 and q == Q - 1),
                        skip_group_check=True)

            for h in range(2):
                for j in range(PAIRS // 2):
                    pair = h * (PAIRS // 2) + j
                    # Big chunk first: its group closes ~50ns earlier on the
                    # last pair, unblocking the ACT drain copy sooner.
                    for k, (c0, cw) in enumerate(CHUNKS):
                        nc.tensor.matmul(
                            reg[h][k],
                            lhsT[:, 2 * pair:2 * pair + 2, :],
                            stile[:, 2 * pair:2 * pair + 2, c0:c0 + cw],
                            start=(j == 0), stop=(j == PAIRS // 2 - 1),
                            perf_mode=mybir.MatmulPerfMode.DoubleRow,
                            skip_group_check=True)
                if h == 1:
                    # gc|gg projections in one region (cons/gen columns are
                    # adjacent in small): sm_in lands right after the stream,
                    # so these run in the post-stream PE window.
                    lhsT32 = small[:, :, 0:G]
                    for nb in range(2):
                        nc.tensor.matmul(pg, lhsT32[:, nb],
                                         small[:, nb, G:],
                                         start=(nb == 0), stop=(nb == 1))
                drain(h)

            # --- merge row + col terms, single out DMA ---
            nc.scalar.copy(out=osb[:, 0:2].rearrange("g k t -> g (k t)"),
                           in_=pg)
            nc.vector.tensor_add(osb[:, 2], rsum, bt)
            nc.sync.dma_start(out=out.rearrange("k g t -> g k t"), in_=osb)
    nc.compile()
    # Drop the framework's const-tensor memsets: nothing reads them, but they
    # run on the Pool engine ahead of the barrier and delay the first SWDGE
    # descriptor emission of the S stream.
    for blk in nc.m.functions[0].blocks:
        blk.instructions = [
            i for i in blk.instructions
            if not (type(i).__name__ == "InstMemset"
                    and i.outs and "const-" in str(i.outs[0]))
        ]
    return nc


_NC_CACHE = None


def _get_program():
    global _NC_CACHE
    if _NC_CACHE is None:
        _NC_CACHE = _build_program()
    return _NC_CACHE


_RUNNER_CACHE = None


def _get_runner():
    """Compiled-once jit(shard_map) executor over 8 cores."""
    global _RUNNER_CACHE
    if _RUNNER_CACHE is None:
        import jax
        from jax.sharding import Mesh, PartitionSpec
        from jax.experimental.shard_map import shard_map
        from concourse import bass2jax, mybir as mb

        nc = _get_program()
        bass2jax.install_neuronx_cc_hook()
        partition_name = (nc.partition_id_tensor.name
                          if nc.partition_id_tensor else None)
        in_names, out_names, out_avals = [], [], []
        for alloc in nc.m.functions[0].allocations:
            if not isinstance(alloc, mb.MemoryLocationSet):
                continue
            name = alloc.memorylocations[0].name
            if alloc.kind == "ExternalInput":
                if name != partition_name:
                    in_names.append(name)
            elif alloc.kind == "ExternalOutput":
                out_names.append(name)
                out_avals.append(jax.core.ShapedArray(
                    tuple(alloc.tensor_shape), mb.dt.np(alloc.dtype)))
        n_params = len(in_names)
        all_names = in_names + out_names
        if partition_name is not None:
            all_names = all_names + [partition_name]

        def _body(*args):
            operands = list(args)
            if partition_name is not None:
                operands.append(bass2jax.partition_id_tensor())
            outs = bass2jax._bass_exec_p.bind(
                *operands,
                out_avals=tuple(out_avals),
                in_names=tuple(all_names),
                out_names=tuple(out_names),
                lowering_input_output_aliases=(),
                sim_require_finite=True,
                sim_require_nnan=True,
                nc=nc,
            )
            return tuple(outs)

        devices = jax.devices()[:8]
        mesh = Mesh(np.asarray(devices), ("core",))
        n_outs = len(out_names)
        sharded = jax.jit(
            shard_map(_body, mesh=mesh,
                      in_specs=(PartitionSpec("core"),) * (n_params + n_outs),
                      out_specs=(PartitionSpec("core"),) * n_outs,
                      check_rep=False),
            donate_argnums=tuple(range(n_params, n_params + n_outs)),
            keep_unused=True,
        )
        _RUNNER_CACHE = (sharded, in_names[:n_params], out_names, out_avals)
    return _RUNNER_CACHE


def _host_side(consumption, generation, sharing_matrix, lv_group_ids,
               valid_lv_mask):
    """Shared input prep: per-core input maps."""
    consumption = np.ascontiguousarray(consumption, dtype=np.float32)
    generation = np.ascontiguousarray(generation, dtype=np.float32)
    sharing_matrix = np.ascontiguousarray(sharing_matrix, dtype=np.float32)
    ids = np.asarray(lv_group_ids)
    valid = np.asarray(valid_lv_mask, dtype=np.float32)

    onehot = (ids[None, :] == np.arange(G)[:, None]).astype(np.float32)
    n_unique = np.float32(np.unique(ids).size)
    M = onehot * valid[None, :]                      # [G, N]
    mt = np.ascontiguousarray(M.T)                   # [N, G]

    # msl[msub, q, g] = -M[g, msub*Q + q]  (negative col weights baked in)
    msl = np.ascontiguousarray((-mt).reshape(C, Q, G).astype(np.float16))

    in_maps = []
    for c in range(8):
        b, hh = divmod(c, 2)
        sl = slice(hh * NLOC, (hh + 1) * NLOC)
        mt_half = mt[sl]                             # [NLOC, G]
        # lhs8[p=(na,msub), blk, :G] = M[g, blk*A + na]; [:, :, G+j] = (msub==j)
        proj = mt_half.reshape(NBLK, A, G).transpose(1, 0, 2)   # [na, blk, g]
        proj = np.broadcast_to(proj[:, None], (A, C, NBLK, G))
        ind = np.broadcast_to(
            np.tile(np.eye(C, dtype=np.float32), (A, 1))[:, None, :],
            (P, NBLK, C))
        lhs = np.concatenate(
            [proj.reshape(P, NBLK, G), ind], axis=2)            # [P, NBLK, GC]
        lhs8 = np.ascontiguousarray(
            lhs.astype(ml_dtypes.float8_e4m3).view(np.uint8))
        # sm_in[p, (nb, f)]: f = [mt row | cons row | gen row] for n = nb*P + p
        sm = np.empty((2, P, G + 2 * T), np.float32)
        sm[:, :, :G] = mt_half.reshape(2, P, G)
        sm[:, :, G:G + T] = consumption[b, sl].reshape(2, P, T)
        sm[:, :, G + T:] = generation[b, sl].reshape(2, P, T)
        in_maps.append({
            "s": np.ascontiguousarray(sharing_matrix[b, sl]),
            "sm_in": np.ascontiguousarray(
                sm.transpose(1, 0, 2).reshape(P, -1)),
            "lhs8": lhs8,
            "msl": msl,
        })
    return in_maps, n_unique


def kernel(consumption, generation, sharing_matrix, lv_group_ids,
           valid_lv_mask, imbalance_penalty_weight, _want_results=False,
           **run_kwargs):
    w = np.float32(np.asarray(imbalance_penalty_weight))
    in_maps, n_unique = _host_side(consumption, generation, sharing_matrix,
                                   lv_group_ids, valid_lv_mask)
    res = None
    if _want_results or run_kwargs:
        nc = _get_program()
        res = run_bass_kernel_spmd(nc, in_maps, core_ids=list(range(8)),
                                   **run_kwargs)
        parts = np.stack([res.results[c]["out"] for c in range(8)])
    else:
        try:
            fn, in_names, out_names, out_avals = _get_runner()
            concat_in = [np.concatenate([m[name] for m in in_maps], axis=0)
                         for name in in_names]
            zeros = [np.zeros((8 * a.shape[0], *a.shape[1:]), a.dtype)
                     for a in out_avals]
            out_arrs = fn(*concat_in, *zeros)
            parts = np.asarray(out_arrs[out_names.index("out")]).reshape(
                8, 3, G, T)
        except Exception:
            nc = _get_program()
            res = run_bass_kernel_spmd(nc, in_maps, core_ids=list(range(8)))
            parts = np.stack([res.results[c]["out"] for c in range(8)])
    full = parts.reshape(B, 2, 3, G, T).sum(axis=1, dtype=np.float32)
    gc, gg, net = full[:, 0], full[:, 1], full[:, 2]

    imbalance = np.abs(gc - gg + net)
    total = gc + gg + np.float32(EPS)
    pen = np.maximum(imbalance / total - np.float32(TOL), np.float32(0))
    outv = np.float32(pen.sum(dtype=np.float32) * w / n_unique)
    out_arr = np.array(outv, dtype=np.float32)
    if _want_results:
        return out_arr, res
    return out_arr



# revision 45
# speedup vs baseline: 1.0538x; 1.0538x over previous
"""Trainium2 Bass kernel for nn_EnergyBalanceChecker (segment_reduce), v5.

Problem (hardcoded): B=4, N=512, T=24, G=32, TOL=0.05, EPS=1e-6.

  M = onehot(lv_group_ids) * valid_lv_mask                     # [G, N]
  gc  = einsum('gn,bnt->bgt', M, consumption)
  gg  = einsum('gn,bnt->bgt', M, generation)
  net = einsum('gn,bnt->bgt', M, S.sum(axis=2) - S.sum(axis=1))
  pen = relu(|gc-gg+net| / (gc+gg+eps) - TOL);  out = pen.sum()*w/n_unique

Sharding: 8 cores = 4 batches x 2 halves of the (row) N axis.

v5 dataflow (vs v4): the q-axis fold moves INTO the matmul pass, so there
is no wide-PSUM drain at all, and the output leaves via a pre-prepared
SWDGE scatter fired by a trigger instruction:
  * S streams in fp8e4 as before (SWDGE cast DMAs; cost is charged on
    destination bytes).  Partitions carry (na in 8 n-rows) x (msub in 16
    m-blocks); free = (q, t).
  * Row term: per (block-pair, q) a DoubleRow matmul with the M[g,n]
    projection lhsT and a T-wide rhs slice accumulates straight into a
    single [G, T] PSUM tile (q and pairs both fold in PSUM).
  * Col term: per (block, q-pair) a DoubleRow matmul pairs two q slices
    of one block with per-q lhsT columns -M[g, m(msub,q)] -- accumulating
    *negated* imports into the SAME [G, T] tile, so net = row - col needs
    no subtract, just one PSUM->SBUF copy at the end.
  * gc|gg from a small f16 side input (one matmul pair, mid-stream).
  * Output: a prepare_only dma_scatter_add writes descriptors during the
    stream; a Pool trigger_dma fires them once the [G,4,T] staging tile is
    ready -- skipping the whole HWDGE SEQ/gen/delay chain at the tail.
  * Host does only the [3, G, T]-level nonlinear tail.
"""

import sys

import numpy as np

try:
    import concourse  # noqa: F401
except ImportError:
    sys.path.insert(0, "/opt/trn_rl_repo")

import ml_dtypes

import concourse.tile as tile
from concourse import bacc, mybir
from concourse.bass_utils import run_bass_kernel_spmd

B, N, T, G = 4, 512, 24, 32
TOL, EPS = 0.05, 1e-6
P = 128                 # SBUF partitions
NLOC = N // 2           # rows per core (n-half)
A = 8                   # n-rows per block (partition sub-dim)
C = 16                  # m-blocks on partitions (partition sub-dim)
Q = N // C              # m-columns per msub block (free dim)
QP = Q // 2             # q-pairs for the col-term matmuls
NBLK = NLOC // A        # 32 blocks of 8 n-rows
PAIRS = NBLK // 2       # DoubleRow pairs
F = Q * T               # free elements per block
DMA_BLOCKS = ((0, 5), (5, 11), (11, 17), (17, 23), (23, 28),
              (28, 30), (30, 32))   # stream DMA block ranges
SMW = G + 2 * T         # smt row: [mt | cons | gen] per n
BLOB_ROWL = P * 0 + NBLK * G           # 1024 u8 per partition
BLOB_SMT = 2 * SMW * 2                 # 320 u8 (f16) per partition
BLOBW = BLOB_ROWL + BLOB_SMT
OW = 3 * T              # out row: [gc | gg | net] per group

_F32 = mybir.dt.float32
_F16 = mybir.dt.float16
_F8 = mybir.dt.float8e4
_U8 = mybir.dt.uint8
_I16 = mybir.dt.int16


def _build_program():
    nc = bacc.Bacc("TRN2", target_bir_lowering=False, debug=False,
                   enable_asserts=False, num_devices=8)
    s = nc.dram_tensor("s", [NLOC, N, T], _F32, kind="ExternalInput").ap()
    blob = nc.dram_tensor("blob", [P, BLOBW], _U8, kind="ExternalInput").ap()
    coln = nc.dram_tensor("coln", [P, QP * 2 * G], _U8,
                          kind="ExternalInput").ap()
    # kv_writeback layout: [batch, d_head_inner, d_head_outer, n_ctx]
    out = nc.dram_tensor("out", [1, P, 1, OW], _F32, kind="ExternalOutput").ap()

    with tile.TileContext(nc) as tc, nc.allow_low_precision(
            "fp8 S stream + fp8 {0,1} masks, f32 PSUM accumulation"):
        with (
            tc.tile_pool(name="sb", bufs=1) as sb,
            tc.tile_pool(name="ps", bufs=1, space="PSUM") as ps,
        ):
            blobt = sb.tile([P, BLOBW], _U8, tag="blobt")
            colnt = sb.tile([P, QP * 2 * G], _U8, tag="colnt")
            stile = sb.tile([P, NBLK, F], _F8, tag="stile")
            # writeback staging: partition p carries [gc_p | gg_p | net_p] as
            # 72 contiguous f32 (partitions 32..127 are zeroed junk the host
            # ignores); kv_writeback streams the whole [128, 72] block out.
            src4 = sb.tile([P, 1, 1, OW], _F32, tag="src")
            src = src4[:, 0, 0]
            ctxi = sb.tile([P, 1], mybir.dt.int32, tag="ctxi")

            # --- small inputs on the SP HWDGE ring: blob lands in the
            # pre-stream DMA window, coln right after stream chunk 1 ---
            nc.sync.dma_start(out=blobt, in_=blob)
            nc.sync.dma_start(out=colnt, in_=coln)

            rowLv = blobt[:, 0:BLOB_ROWL].bitcast(_F8).rearrange(
                "p (b g) -> p b g", b=NBLK)
            smtv = blobt[:, BLOB_ROWL:BLOB_ROWL + BLOB_SMT].bitcast(
                _F16).rearrange("p (nb f) -> p nb f", nb=2)
            colNv = colnt.bitcast(_F8).rearrange(
                "p (j k g) -> p j k g", j=QP, k=2)

            # three banks, all at partition base 0
            gcp = ps.tile([G, T], _F32, tag="gcp")
            ggp = ps.tile([G, T], _F32, tag="ggp")
            netp = ps.tile([G, T], _F32, tag="netp")
            nc.vector.memset(src[:], 0.0)
            nc.vector.memset(ctxi[:], 0)

            # --- S stream: fp8 cast DMAs on the SWDGE ring ---
            # partition p = na*C + msub; block blk: n = blk*A + na;
            # free = (q, t) with m = msub*Q + q.
            s_r = s.rearrange("(blk a) (c q) t -> (a c) blk (q t)", a=A, c=C)
            for b0, b1 in DMA_BLOCKS:
                nc.gpsimd.dma_start(
                    out=stile[:, b0:b1, :].rearrange("p b f -> p (b f)"),
                    in_=s_r[:, b0:b1, :])

            # --- output path: descriptors prepared during the stream, the
            # trigger fires them once `src` is fully written ---
            dma_sem = nc.alloc_semaphore("outdma")
            nc.gpsimd.kv_writeback(
                out, src4[:], ctxi[:], prepare_only=True, sem=dma_sem)

            # --- PE pass ---
            # gc|gg projections first in PE program order: smt arrives with
            # the blob (~2us), well before the first S pair is consumable.
            for nb in range(2):
                nc.tensor.matmul(gcp, smtv[:, nb, 0:G],
                                 smtv[:, nb, G:G + T],
                                 start=(nb == 0), stop=(nb == 1),
                                 skip_group_check=True)
                nc.tensor.matmul(ggp, smtv[:, nb, 0:G],
                                 smtv[:, nb, G + T:],
                                 start=(nb == 0), stop=(nb == 1),
                                 skip_group_check=True)

            # One [G, T] accumulation group over all 1024 DoubleRow matmuls:
            # row term adds M[g,n]-projected q-slices (2 blocks per pass),
            # col term adds -M[g,m]-weighted q-pairs (2 q per pass).
            netw = netp
            for pr in range(PAIRS):
                lhs_row = rowLv[:, 2 * pr:2 * pr + 2, :]
                for q in range(Q):
                    nc.tensor.matmul(
                        netw, lhs_row,
                        stile[:, 2 * pr:2 * pr + 2, q * T:(q + 1) * T],
                        start=(pr == 0 and q == 0), stop=False,
                        perf_mode=mybir.MatmulPerfMode.DoubleRow,
                        skip_group_check=True)
                for blk in (2 * pr, 2 * pr + 1):
                    for j in range(QP):
                        nc.tensor.matmul(
                            netw, colNv[:, j],
                            stile[:, blk, 2 * j * T:(2 * j + 2) * T]
                            .rearrange("p (k t) -> p k t", k=2),
                            start=False,
                            stop=(pr == PAIRS - 1 and blk == 2 * pr + 1
                                  and j == QP - 1),
                            perf_mode=mybir.MatmulPerfMode.DoubleRow,
                            skip_group_check=True)

            # --- stage [gc | gg | net] and fire the scatter ---
            # Partition-preserving ACT copies; only the net copy is on the
            # post-stream critical path.
            act_done = nc.alloc_semaphore("actdone")
            nc.scalar.copy(out=src[0:G, 0:T], in_=gcp)
            nc.scalar.copy(out=src[0:G, T:2 * T], in_=ggp)
            nc.scalar.copy(out=src[0:G, 2 * T:3 * T], in_=netp)
            # Placeholder gate (>=0 so the schedule-time sim sails through);
            # post-compile surgery points it at the tile Activation engine sem
            # (ACT instructions cannot carry a second sync update, and the
            # trigger cannot carry a second wait).
            nc.gpsimd.wait_ge(act_done, 0)
            nc.gpsimd.trigger_dma(count=None)
    nc.compile()
    # Drop the framework's const-tensor memsets: nothing reads them, but they
    # run on the Pool engine ahead of the barrier and delay the first SWDGE
    # descriptor emission of the S stream.
    for blk in nc.m.functions[0].blocks:
        blk.instructions = [
            i for i in blk.instructions
            if not (type(i).__name__ == "InstMemset"
                    and i.outs and "const-" in str(i.outs[0]))
        ]
    # Tile schedules the scatter prep on a DMASW lane and the exit drain
    # waits on that lane's sem, but the descriptor-baked completion sem
    # (on_update[0], hardware increments by 16) is the user sem= kwarg.
    # Point on_update[0] at the orphaned DMASW sem so the DMA engines bump
    # the sem the drain actually waits on.
    fn = nc.m.functions[0]
    updated, waited, prep = set(), {}, None
    for blk in fn.blocks:
        for ins in blk.instructions:
            if type(ins).__name__ == "InstKVWritebackAnt":
                prep = ins
            si = ins.sync_info
            if si is None:
                continue
            for u in si.on_update:
                updated.add(u.id)
            for w in si.on_wait:
                waited[w.id] = w
    orphans = [w for wid, w in waited.items()
               if wid not in updated and (w.ant_name or "").startswith("DMASW")]
    assert prep is not None and len(orphans) == 1, (prep, orphans)
    u0 = prep.sync_info.on_update[0]
    assert u0.ant_name == "outdma", u0
    prep.sync_info.on_update[0] = mybir.SyncUpdate(
        sync_type=u0.sync_type, id=orphans[0].id, ant_name=orphans[0].ant_name,
        update_mode=u0.update_mode, update_value=u0.update_value,
        update_reg=u0.update_reg)
    # The sem-assignment pass drops the trigger's cross-engine RAW waits (it
    # only gates on the prep's Pool tick), so the trigger could fire before
    # the staging copies.  The placeholder wait_ge(actdone) sits right before
    # the trigger on the Pool SEQ; point it at the Activation engine-proc sem
    # with the cumulative tick of the last staging copy.
    # The scheduler can linearize the ACT exit drain (which waits on the
    # writeback's DMASW sem) BEFORE the staging copies on the same engine --
    # circular in strict block order.  Move the copies ahead of any
    # instruction waiting on the orphan sem.
    orphan_id = orphans[0].id
    for blk in fn.blocks:
        insts = blk.instructions
        drain_pos = None
        for i, ins in enumerate(insts):
            si = ins.sync_info
            if si and any(w.id == orphan_id for w in si.on_wait):
                drain_pos = i
                break
        if drain_pos is None:
            continue
        late = [ins for ins in insts[drain_pos:]
                if type(ins).__name__ == "InstActivation"]
        if late:
            rest = [ins for ins in insts if ins not in late]
            blk.instructions = (rest[:drain_pos] + late + rest[drain_pos:])
    act_total = 0
    last_src_tick = None
    gate = None
    for blk in fn.blocks:
        for ins in blk.instructions:
            si = ins.sync_info
            if si is None:
                continue
            for w in si.on_wait:
                if w.ant_name == "actdone":
                    gate = ins
            for u in si.on_update:
                if (u.ant_name or "").startswith("Activation_"):
                    act_total += (u.update_value or 1)
                    if type(ins).__name__ == "InstActivation":
                        last_src_tick = (u.id, u.ant_name, act_total)
    assert gate is not None and last_src_tick is not None, (gate, last_src_tick)
    sid, sname, val = last_src_tick
    gate.sync_info.on_wait = [mybir.SyncWait(
        sync_type="semaphore", id=sid, ant_name=sname,
        wait_mode="sem-ge-imm", wait_value=val, wait_reg=None)]
    return nc


_NC_CACHE = None


def _get_program():
    global _NC_CACHE
    if _NC_CACHE is None:
        _NC_CACHE = _build_program()
    return _NC_CACHE


_RUNNER_CACHE = None


def _get_runner():
    """Compiled-once jit(shard_map) executor over 8 cores."""
    global _RUNNER_CACHE
    if _RUNNER_CACHE is None:
        import jax
        from jax.sharding import Mesh, PartitionSpec
        from jax.experimental.shard_map import shard_map
        from concourse import bass2jax, mybir as mb

        nc = _get_program()
        bass2jax.install_neuronx_cc_hook()
        partition_name = (nc.partition_id_tensor.name
                          if nc.partition_id_tensor else None)
        in_names, out_names, out_avals = [], [], []
        for alloc in nc.m.functions[0].allocations:
            if not isinstance(alloc, mb.MemoryLocationSet):
                continue
            name = alloc.memorylocations[0].name
            if alloc.kind == "ExternalInput":
                if name != partition_name:
                    in_names.append(name)
            elif alloc.kind == "ExternalOutput":
                out_names.append(name)
                out_avals.append(jax.core.ShapedArray(
                    tuple(alloc.tensor_shape), mb.dt.np(alloc.dtype)))
        n_params = len(in_names)
        all_names = in_names + out_names
        if partition_name is not None:
            all_names = all_names + [partition_name]

        def _body(*args):
            operands = list(args)
            if partition_name is not None:
                operands.append(bass2jax.partition_id_tensor())
            outs = bass2jax._bass_exec_p.bind(
                *operands,
                out_avals=tuple(out_avals),
                in_names=tuple(all_names),
                out_names=tuple(out_names),
                lowering_input_output_aliases=(),
                sim_require_finite=True,
                sim_require_nnan=True,
                nc=nc,
            )
            return tuple(outs)

        devices = jax.devices()[:8]
        mesh = Mesh(np.asarray(devices), ("core",))
        n_outs = len(out_names)
        sharded = jax.jit(
            shard_map(_body, mesh=mesh,
                      in_specs=(PartitionSpec("core"),) * (n_params + n_outs),
                      out_specs=(PartitionSpec("core"),) * n_outs,
                      check_rep=False),
            donate_argnums=tuple(range(n_params, n_params + n_outs)),
            keep_unused=True,
        )
        _RUNNER_CACHE = (sharded, in_names[:n_params], out_names, out_avals)
    return _RUNNER_CACHE


def _host_side(consumption, generation, sharing_matrix, lv_group_ids,
               valid_lv_mask):
    """Shared input prep: per-core input maps."""
    consumption = np.ascontiguousarray(consumption, dtype=np.float32)
    generation = np.ascontiguousarray(generation, dtype=np.float32)
    sharing_matrix = np.ascontiguousarray(sharing_matrix, dtype=np.float32)
    ids = np.asarray(lv_group_ids)
    valid = np.asarray(valid_lv_mask, dtype=np.float32)

    onehot = (ids[None, :] == np.arange(G)[:, None]).astype(np.float32)
    n_unique = np.float32(np.unique(ids).size)
    M = onehot * valid[None, :]                      # [G, N]
    mt = np.ascontiguousarray(M.T)                   # [N, G]

    # coln[p=(na,msub), j, k, g] = -M[g, msub*Q + 2j + k]  (negated imports)
    cw = (-mt).reshape(C, Q, G)                      # [msub, q, g]
    coln = np.broadcast_to(cw[None], (A, C, Q, G)).reshape(P, Q * G)
    coln8 = np.ascontiguousarray(
        coln.astype(ml_dtypes.float8_e4m3).view(np.uint8))

    in_maps = []
    for c in range(8):
        b, hh = divmod(c, 2)
        sl = slice(hh * NLOC, (hh + 1) * NLOC)
        mt_half = mt[sl]                             # [NLOC, G]
        # rowL[p=(na,msub), blk, g] = M[g, blk*A + na]
        proj = mt_half.reshape(NBLK, A, G).transpose(1, 0, 2)   # [na, blk, g]
        proj = np.broadcast_to(proj[:, None], (A, C, NBLK, G))
        rowl8 = proj.reshape(P, NBLK * G).astype(
            ml_dtypes.float8_e4m3).view(np.uint8)
        # smt[p, nb, f]: f = [mt row | cons row | gen row] for n = nb*P + p
        sm = np.empty((2, P, SMW), np.float16)
        sm[:, :, :G] = mt_half.reshape(2, P, G)
        sm[:, :, G:G + T] = consumption[b, sl].reshape(2, P, T)
        sm[:, :, G + T:] = generation[b, sl].reshape(2, P, T)
        sm_bytes = sm.transpose(1, 0, 2).reshape(P, -1).view(np.uint8)

        blob = np.zeros((P, BLOBW), np.uint8)
        blob[:, :BLOB_ROWL] = rowl8
        blob[:, BLOB_ROWL:BLOB_ROWL + BLOB_SMT] = sm_bytes
        in_maps.append({
            "s": np.ascontiguousarray(sharing_matrix[b, sl]),
            "blob": np.ascontiguousarray(blob),
            "coln": coln8,
        })
    return in_maps, n_unique


def kernel(consumption, generation, sharing_matrix, lv_group_ids,
           valid_lv_mask, imbalance_penalty_weight, _want_results=False,
           **run_kwargs):
    w = np.float32(np.asarray(imbalance_penalty_weight))
    in_maps, n_unique = _host_side(consumption, generation, sharing_matrix,
                                   lv_group_ids, valid_lv_mask)
    res = None
    if _want_results or run_kwargs:
        nc = _get_program()
        res = run_bass_kernel_spmd(nc, in_maps, core_ids=list(range(8)),
                                   **run_kwargs)
        parts = np.stack([res.results[c]["out"] for c in range(8)])
    else:
        try:
            fn, in_names, out_names, out_avals = _get_runner()
            concat_in = [np.concatenate([m[name] for m in in_maps], axis=0)
                         for name in in_names]
            zeros = [np.zeros((8 * a.shape[0], *a.shape[1:]), a.dtype)
                     for a in out_avals]
            out_arrs = fn(*concat_in, *zeros)
            parts = np.asarray(out_arrs[out_names.index("out")]).reshape(
                8, P, OW)
        except Exception:
            nc = _get_program()
            res = run_bass_kernel_spmd(nc, in_maps, core_ids=list(range(8)))
            parts = np.stack([res.results[c]["out"] for c in range(8)]).reshape(
                8, P, OW)
    # partition p (< G) carries [gc_p | gg_p | net_p] as 3*T columns
    per_core = parts[:, :G, :].reshape(8, G, 3, T).transpose(0, 2, 1, 3)
    full = per_core.reshape(B, 2, 3, G, T).sum(axis=1, dtype=np.float32)
    gc, gg, net = full[:, 0], full[:, 1], full[:, 2]

    imbalance = np.abs(gc - gg + net)
    total = gc + gg + np.float32(EPS)
    pen = np.maximum(imbalance / total - np.float32(TOL), np.float32(0))
    outv = np.float32(pen.sum(dtype=np.float32) * w / n_unique)
    out_arr = np.array(outv, dtype=np.float32)
    if _want_results:
        return out_arr, res
    return out_arr


# revision 46
# speedup vs baseline: 1.1385x; 1.0804x over previous
"""Trainium2 Bass kernel for nn_EnergyBalanceChecker (segment_reduce), v5.

Problem (hardcoded): B=4, N=512, T=24, G=32, TOL=0.05, EPS=1e-6.

  M = onehot(lv_group_ids) * valid_lv_mask                     # [G, N]
  gc  = einsum('gn,bnt->bgt', M, consumption)
  gg  = einsum('gn,bnt->bgt', M, generation)
  net = einsum('gn,bnt->bgt', M, S.sum(axis=2) - S.sum(axis=1))
  pen = relu(|gc-gg+net| / (gc+gg+eps) - TOL);  out = pen.sum()*w/n_unique

Sharding: 8 cores = 4 batches x 2 halves of the (row) N axis.

v5 dataflow (vs v4): the q-axis fold moves INTO the matmul pass, so there
is no wide-PSUM drain at all, and the output leaves via a pre-prepared
SWDGE scatter fired by a trigger instruction:
  * S streams in fp8e4 as before (SWDGE cast DMAs; cost is charged on
    destination bytes).  Partitions carry (na in 8 n-rows) x (msub in 16
    m-blocks); free = (q, t).
  * Row term: per (block-pair, q) a DoubleRow matmul with the M[g,n]
    projection lhsT and a T-wide rhs slice accumulates straight into a
    single [G, T] PSUM tile (q and pairs both fold in PSUM).
  * Col term: per (block, q-pair) a DoubleRow matmul pairs two q slices
    of one block with per-q lhsT columns -M[g, m(msub,q)] -- accumulating
    *negated* imports into the SAME [G, T] tile, so net = row - col needs
    no subtract, just one PSUM->SBUF copy at the end.
  * gc|gg from a small f16 side input (one matmul pair, mid-stream).
  * Output: a prepare_only dma_scatter_add writes descriptors during the
    stream; a Pool trigger_dma fires them once the [G,4,T] staging tile is
    ready -- skipping the whole HWDGE SEQ/gen/delay chain at the tail.
  * Host does only the [3, G, T]-level nonlinear tail.
"""

import sys

import numpy as np

try:
    import concourse  # noqa: F401
except ImportError:
    sys.path.insert(0, "/opt/trn_rl_repo")

import ml_dtypes

import concourse.tile as tile
from concourse import bacc, mybir
from concourse.bass_utils import run_bass_kernel_spmd

B, N, T, G = 4, 512, 24, 32
TOL, EPS = 0.05, 1e-6
P = 128                 # SBUF partitions
NLOC = N // 2           # rows per core (n-half)
A = 8                   # n-rows per block (partition sub-dim)
C = 16                  # m-blocks on partitions (partition sub-dim)
Q = N // C              # m-columns per msub block (free dim)
QP = Q // 2             # q-pairs for the col-term matmuls
NBLK = NLOC // A        # 32 blocks of 8 n-rows
PAIRS = NBLK // 2       # DoubleRow pairs
F = Q * T               # free elements per block
DMA_BLOCKS = ((0, 5), (5, 11), (11, 17), (17, 23), (23, 28),
              (28, 30), (30, 32))   # stream DMA block ranges
SMW = G + 2 * T         # smt row: [mt | cons | gen] per n
BLOB_ROWL = P * 0 + NBLK * G           # 1024 u8 per partition
BLOB_SMT = 2 * SMW * 2                 # 320 u8 (f16) per partition
BLOBW = BLOB_ROWL + BLOB_SMT
OW = 3 * T              # out row: [gc | gg | net] per group

_F32 = mybir.dt.float32
_F16 = mybir.dt.float16
_F8 = mybir.dt.float8e4
_U8 = mybir.dt.uint8
_I16 = mybir.dt.int16


def _build_program():
    nc = bacc.Bacc("TRN2", target_bir_lowering=False, debug=False,
                   enable_asserts=False, num_devices=8)
    s = nc.dram_tensor("s", [NLOC, N, T], _F32, kind="ExternalInput").ap()
    blob = nc.dram_tensor("blob", [P, BLOBW], _U8, kind="ExternalInput").ap()
    coln = nc.dram_tensor("coln", [P, QP * 2 * G], _U8,
                          kind="ExternalInput").ap()
    # kv_writeback layout: [batch, d_head_inner, d_head_outer, n_ctx]
    out = nc.dram_tensor("out", [1, P, 1, OW], _F32, kind="ExternalOutput").ap()

    with tile.TileContext(nc) as tc, nc.allow_low_precision(
            "fp8 S stream + fp8 {0,1} masks, f32 PSUM accumulation"):
        with (
            tc.tile_pool(name="sb", bufs=1) as sb,
            tc.tile_pool(name="ps", bufs=1, space="PSUM") as ps,
        ):
            blobt = sb.tile([P, BLOBW], _U8, tag="blobt")
            colnt = sb.tile([P, QP * 2 * G], _U8, tag="colnt")
            stile = sb.tile([P, NBLK, F], _F8, tag="stile")
            # writeback staging: partition p carries [gc_p | gg_p | net_p] as
            # 72 contiguous f32 (partitions 32..127 are zeroed junk the host
            # ignores); kv_writeback streams the whole [128, 72] block out.
            src4 = sb.tile([P, 1, 1, OW], _F32, tag="src")
            src = src4[:, 0, 0]
            ctxi = sb.tile([P, 1], mybir.dt.int32, tag="ctxi")

            # --- small inputs on the SP HWDGE ring: blob lands in the
            # pre-stream DMA window, coln right after stream chunk 1 ---
            nc.sync.dma_start(out=blobt, in_=blob)
            nc.sync.dma_start(out=colnt, in_=coln)

            rowLv = blobt[:, 0:BLOB_ROWL].bitcast(_F8).rearrange(
                "p (b g) -> p b g", b=NBLK)
            smtv = blobt[:, BLOB_ROWL:BLOB_ROWL + BLOB_SMT].bitcast(
                _F16).rearrange("p (nb f) -> p nb f", nb=2)
            colNv = colnt.bitcast(_F8).rearrange(
                "p (j k g) -> p j k g", j=QP, k=2)

            # three banks, all at partition base 0
            gcp = ps.tile([G, T], _F32, tag="gcp")
            ggp = ps.tile([G, T], _F32, tag="ggp")
            netp = ps.tile([G, T], _F32, tag="netp")
            nc.vector.memset(src[:], 0.0)
            nc.vector.memset(ctxi[:], 0)

            # --- S stream: fp8 cast DMAs on the SWDGE ring ---
            # partition p = na*C + msub; block blk: n = blk*A + na;
            # free = (q, t) with m = msub*Q + q.
            s_r = s.rearrange("(blk a) (c q) t -> (a c) blk (q t)", a=A, c=C)
            for b0, b1 in DMA_BLOCKS:
                nc.gpsimd.dma_start(
                    out=stile[:, b0:b1, :].rearrange("p b f -> p (b f)"),
                    in_=s_r[:, b0:b1, :])

            # --- output path: descriptors prepared during the stream, the
            # trigger fires them once `src` is fully written ---
            dma_sem = nc.alloc_semaphore("outdma")
            nc.gpsimd.kv_writeback(
                out, src4[:], ctxi[:], prepare_only=True, sem=dma_sem)

            # --- PE pass ---
            # gc|gg projections first in PE program order: smt arrives with
            # the blob (~2us), well before the first S pair is consumable.
            for nb in range(2):
                nc.tensor.matmul(gcp, smtv[:, nb, 0:G],
                                 smtv[:, nb, G:G + T],
                                 start=(nb == 0), stop=(nb == 1),
                                 skip_group_check=True)
                nc.tensor.matmul(ggp, smtv[:, nb, 0:G],
                                 smtv[:, nb, G + T:],
                                 start=(nb == 0), stop=(nb == 1),
                                 skip_group_check=True)

            # One [G, T] accumulation group over all 1024 DoubleRow matmuls:
            # row term adds M[g,n]-projected q-slices (2 blocks per pass),
            # col term adds -M[g,m]-weighted q-pairs (2 q per pass).
            netw = netp
            for pr in range(PAIRS):
                lhs_row = rowLv[:, 2 * pr:2 * pr + 2, :]
                for q in range(Q):
                    nc.tensor.matmul(
                        netw, lhs_row,
                        stile[:, 2 * pr:2 * pr + 2, q * T:(q + 1) * T],
                        start=(pr == 0 and q == 0), stop=False,
                        perf_mode=mybir.MatmulPerfMode.DoubleRow,
                        skip_group_check=True)
                for blk in (2 * pr, 2 * pr + 1):
                    for j in range(QP):
                        nc.tensor.matmul(
                            netw, colNv[:, j],
                            stile[:, blk, 2 * j * T:(2 * j + 2) * T]
                            .rearrange("p (k t) -> p k t", k=2),
                            start=False,
                            stop=(pr == PAIRS - 1 and blk == 2 * pr + 1
                                  and j == QP - 1),
                            perf_mode=mybir.MatmulPerfMode.DoubleRow,
                            skip_group_check=True)

            # --- stage [gc | gg | net] and fire the scatter ---
            # Partition-preserving ACT copies; only the net copy is on the
            # post-stream critical path.
            act_done = nc.alloc_semaphore("actdone")
            nc.scalar.copy(out=src[0:G, 0:T], in_=gcp)
            nc.scalar.copy(out=src[0:G, T:2 * T], in_=ggp)
            nc.scalar.copy(out=src[0:G, 2 * T:3 * T], in_=netp)
            # Placeholder gate (>=0 so the schedule-time sim sails through);
            # post-compile surgery points it at the tile Activation engine sem
            # (ACT instructions cannot carry a second sync update, and the
            # trigger cannot carry a second wait).
            nc.gpsimd.wait_ge(act_done, 0)
            nc.gpsimd.trigger_dma(count=None)
    nc.compile()
    # Drop the framework's const-tensor memsets: nothing reads them, but they
    # run on the Pool engine ahead of the barrier and delay the first SWDGE
    # descriptor emission of the S stream.
    for blk in nc.m.functions[0].blocks:
        blk.instructions = [
            i for i in blk.instructions
            if not (type(i).__name__ == "InstMemset"
                    and i.outs and "const-" in str(i.outs[0]))
        ]
    # Tile schedules the scatter prep on a DMASW lane and the exit drain
    # waits on that lane's sem, but the descriptor-baked completion sem
    # (on_update[0], hardware increments by 16) is the user sem= kwarg.
    # Point on_update[0] at the orphaned DMASW sem so the DMA engines bump
    # the sem the drain actually waits on.
    fn = nc.m.functions[0]
    updated, waited, prep = set(), {}, None
    for blk in fn.blocks:
        for ins in blk.instructions:
            if type(ins).__name__ == "InstKVWritebackAnt":
                prep = ins
            si = ins.sync_info
            if si is None:
                continue
            for u in si.on_update:
                updated.add(u.id)
            for w in si.on_wait:
                waited[w.id] = w
    orphans = [w for wid, w in waited.items()
               if wid not in updated and (w.ant_name or "").startswith("DMASW")]
    assert prep is not None and len(orphans) == 1, (prep, orphans)
    u0 = prep.sync_info.on_update[0]
    assert u0.ant_name == "outdma", u0
    prep.sync_info.on_update[0] = mybir.SyncUpdate(
        sync_type=u0.sync_type, id=orphans[0].id, ant_name=orphans[0].ant_name,
        update_mode=u0.update_mode, update_value=u0.update_value,
        update_reg=u0.update_reg)
    # The sem-assignment pass drops the trigger's cross-engine RAW waits (it
    # only gates on the prep's Pool tick), so the trigger could fire before
    # the staging copies.  The placeholder wait_ge(actdone) sits right before
    # the trigger on the Pool SEQ; point it at the Activation engine-proc sem
    # with the cumulative tick of the last staging copy.
    # The scheduler can linearize the ACT exit drain (which waits on the
    # writeback's DMASW sem) BEFORE the staging copies on the same engine --
    # circular in strict block order.  Move the copies ahead of any
    # instruction waiting on the orphan sem.
    orphan_id = orphans[0].id
    for blk in fn.blocks:
        insts = blk.instructions
        drain_pos = None
        for i, ins in enumerate(insts):
            si = ins.sync_info
            if si and any(w.id == orphan_id for w in si.on_wait):
                drain_pos = i
                break
        if drain_pos is None:
            continue
        late = [ins for ins in insts[drain_pos:]
                if type(ins).__name__ == "InstActivation"]
        if late:
            rest = [ins for ins in insts if ins not in late]
            blk.instructions = (rest[:drain_pos] + late + rest[drain_pos:])
    # The framework's ACT table load lands in the postamble AFTER the exit
    # wait on the writeback sem, adding ~1.3us of pure tail.  Hoist it to the
    # head of the main block so it overlaps the stream (baseline behavior).
    loads = []
    for blk in fn.blocks:
        keep = []
        for ins in blk.instructions:
            if type(ins).__name__ == "InstLoadActFuncSet":
                loads.append(ins)
            else:
                keep.append(ins)
        blk.instructions = keep
    if loads:
        main = fn.blocks[1]
        main.instructions = loads + main.instructions
    act_total = 0
    last_src_tick = None
    gate = None
    for blk in fn.blocks:
        for ins in blk.instructions:
            si = ins.sync_info
            if si is None:
                continue
            for w in si.on_wait:
                if w.ant_name == "actdone":
                    gate = ins
            for u in si.on_update:
                if (u.ant_name or "").startswith("Activation_"):
                    act_total += (u.update_value or 1)
                    if type(ins).__name__ == "InstActivation":
                        last_src_tick = (u.id, u.ant_name, act_total)
    assert gate is not None and last_src_tick is not None, (gate, last_src_tick)
    sid, sname, val = last_src_tick
    gate.sync_info.on_wait = [mybir.SyncWait(
        sync_type="semaphore", id=sid, ant_name=sname,
        wait_mode="sem-ge-imm", wait_value=val, wait_reg=None)]
    return nc


_NC_CACHE = None


def _get_program():
    global _NC_CACHE
    if _NC_CACHE is None:
        _NC_CACHE = _build_program()
    return _NC_CACHE


_RUNNER_CACHE = None


def _get_runner():
    """Compiled-once jit(shard_map) executor over 8 cores."""
    global _RUNNER_CACHE
    if _RUNNER_CACHE is None:
        import jax
        from jax.sharding import Mesh, PartitionSpec
        from jax.experimental.shard_map import shard_map
        from concourse import bass2jax, mybir as mb

        nc = _get_program()
        bass2jax.install_neuronx_cc_hook()
        partition_name = (nc.partition_id_tensor.name
                          if nc.partition_id_tensor else None)
        in_names, out_names, out_avals = [], [], []
        for alloc in nc.m.functions[0].allocations:
            if not isinstance(alloc, mb.MemoryLocationSet):
                continue
            name = alloc.memorylocations[0].name
            if alloc.kind == "ExternalInput":
                if name != partition_name:
                    in_names.append(name)
            elif alloc.kind == "ExternalOutput":
                out_names.append(name)
                out_avals.append(jax.core.ShapedArray(
                    tuple(alloc.tensor_shape), mb.dt.np(alloc.dtype)))
        n_params = len(in_names)
        all_names = in_names + out_names
        if partition_name is not None:
            all_names = all_names + [partition_name]

        def _body(*args):
            operands = list(args)
            if partition_name is not None:
                operands.append(bass2jax.partition_id_tensor())
            outs = bass2jax._bass_exec_p.bind(
                *operands,
                out_avals=tuple(out_avals),
                in_names=tuple(all_names),
                out_names=tuple(out_names),
                lowering_input_output_aliases=(),
                sim_require_finite=True,
                sim_require_nnan=True,
                nc=nc,
            )
            return tuple(outs)

        devices = jax.devices()[:8]
        mesh = Mesh(np.asarray(devices), ("core",))
        n_outs = len(out_names)
        sharded = jax.jit(
            shard_map(_body, mesh=mesh,
                      in_specs=(PartitionSpec("core"),) * (n_params + n_outs),
                      out_specs=(PartitionSpec("core"),) * n_outs,
                      check_rep=False),
            donate_argnums=tuple(range(n_params, n_params + n_outs)),
            keep_unused=True,
        )
        _RUNNER_CACHE = (sharded, in_names[:n_params], out_names, out_avals)
    return _RUNNER_CACHE


def _host_side(consumption, generation, sharing_matrix, lv_group_ids,
               valid_lv_mask):
    """Shared input prep: per-core input maps."""
    consumption = np.ascontiguousarray(consumption, dtype=np.float32)
    generation = np.ascontiguousarray(generation, dtype=np.float32)
    sharing_matrix = np.ascontiguousarray(sharing_matrix, dtype=np.float32)
    ids = np.asarray(lv_group_ids)
    valid = np.asarray(valid_lv_mask, dtype=np.float32)

    onehot = (ids[None, :] == np.arange(G)[:, None]).astype(np.float32)
    n_unique = np.float32(np.unique(ids).size)
    M = onehot * valid[None, :]                      # [G, N]
    mt = np.ascontiguousarray(M.T)                   # [N, G]

    # coln[p=(na,msub), j, k, g] = -M[g, msub*Q + 2j + k]  (negated imports)
    cw = (-mt).reshape(C, Q, G)                      # [msub, q, g]
    coln = np.broadcast_to(cw[None], (A, C, Q, G)).reshape(P, Q * G)
    coln8 = np.ascontiguousarray(
        coln.astype(ml_dtypes.float8_e4m3).view(np.uint8))

    in_maps = []
    for c in range(8):
        b, hh = divmod(c, 2)
        sl = slice(hh * NLOC, (hh + 1) * NLOC)
        mt_half = mt[sl]                             # [NLOC, G]
        # rowL[p=(na,msub), blk, g] = M[g, blk*A + na]
        proj = mt_half.reshape(NBLK, A, G).transpose(1, 0, 2)   # [na, blk, g]
        proj = np.broadcast_to(proj[:, None], (A, C, NBLK, G))
        rowl8 = proj.reshape(P, NBLK * G).astype(
            ml_dtypes.float8_e4m3).view(np.uint8)
        # smt[p, nb, f]: f = [mt row | cons row | gen row] for n = nb*P + p
        sm = np.empty((2, P, SMW), np.float16)
        sm[:, :, :G] = mt_half.reshape(2, P, G)
        sm[:, :, G:G + T] = consumption[b, sl].reshape(2, P, T)
        sm[:, :, G + T:] = generation[b, sl].reshape(2, P, T)
        sm_bytes = sm.transpose(1, 0, 2).reshape(P, -1).view(np.uint8)

        blob = np.zeros((P, BLOBW), np.uint8)
        blob[:, :BLOB_ROWL] = rowl8
        blob[:, BLOB_ROWL:BLOB_ROWL + BLOB_SMT] = sm_bytes
        in_maps.append({
            "s": np.ascontiguousarray(sharing_matrix[b, sl]),
            "blob": np.ascontiguousarray(blob),
            "coln": coln8,
        })
    return in_maps, n_unique


def kernel(consumption, generation, sharing_matrix, lv_group_ids,
           valid_lv_mask, imbalance_penalty_weight, _want_results=False,
           **run_kwargs):
    w = np.float32(np.asarray(imbalance_penalty_weight))
    in_maps, n_unique = _host_side(consumption, generation, sharing_matrix,
                                   lv_group_ids, valid_lv_mask)
    res = None
    if _want_results or run_kwargs:
        nc = _get_program()
        res = run_bass_kernel_spmd(nc, in_maps, core_ids=list(range(8)),
                                   **run_kwargs)
        parts = np.stack([res.results[c]["out"] for c in range(8)])
    else:
        try:
            fn, in_names, out_names, out_avals = _get_runner()
            concat_in = [np.concatenate([m[name] for m in in_maps], axis=0)
                         for name in in_names]
            zeros = [np.zeros((8 * a.shape[0], *a.shape[1:]), a.dtype)
                     for a in out_avals]
            out_arrs = fn(*concat_in, *zeros)
            parts = np.asarray(out_arrs[out_names.index("out")]).reshape(
                8, P, OW)
        except Exception:
            nc = _get_program()
            res = run_bass_kernel_spmd(nc, in_maps, core_ids=list(range(8)))
            parts = np.stack([res.results[c]["out"] for c in range(8)]).reshape(
                8, P, OW)
    # partition p (< G) carries [gc_p | gg_p | net_p] as 3*T columns
    per_core = parts[:, :G, :].reshape(8, G, 3, T).transpose(0, 2, 1, 3)
    full = per_core.reshape(B, 2, 3, G, T).sum(axis=1, dtype=np.float32)
    gc, gg, net = full[:, 0], full[:, 1], full[:, 2]

    imbalance = np.abs(gc - gg + net)
    total = gc + gg + np.float32(EPS)
    pen = np.maximum(imbalance / total - np.float32(TOL), np.float32(0))
    outv = np.float32(pen.sum(dtype=np.float32) * w / n_unique)
    out_arr = np.array(outv, dtype=np.float32)
    if _want_results:
        return out_arr, res
    return out_arr


# revision 52
# speedup vs baseline: 1.1787x; 1.0353x over previous
"""Trainium2 Bass kernel for nn_EnergyBalanceChecker (segment_reduce), v5.

Problem (hardcoded): B=4, N=512, T=24, G=32, TOL=0.05, EPS=1e-6.

  M = onehot(lv_group_ids) * valid_lv_mask                     # [G, N]
  gc  = einsum('gn,bnt->bgt', M, consumption)
  gg  = einsum('gn,bnt->bgt', M, generation)
  net = einsum('gn,bnt->bgt', M, S.sum(axis=2) - S.sum(axis=1))
  pen = relu(|gc-gg+net| / (gc+gg+eps) - TOL);  out = pen.sum()*w/n_unique

Sharding: 8 cores = 4 batches x 2 halves of the (row) N axis.

v5 dataflow (vs v4): the q-axis fold moves INTO the matmul pass, so there
is no wide-PSUM drain at all, and the output leaves via a pre-prepared
SWDGE scatter fired by a trigger instruction:
  * S streams in fp8e4 as before (SWDGE cast DMAs; cost is charged on
    destination bytes).  Partitions carry (na in 8 n-rows) x (msub in 16
    m-blocks); free = (q, t).
  * Row term: per (block-pair, q) a DoubleRow matmul with the M[g,n]
    projection lhsT and a T-wide rhs slice accumulates straight into a
    single [G, T] PSUM tile (q and pairs both fold in PSUM).
  * Col term: per (block, q-pair) a DoubleRow matmul pairs two q slices
    of one block with per-q lhsT columns -M[g, m(msub,q)] -- accumulating
    *negated* imports into the SAME [G, T] tile, so net = row - col needs
    no subtract, just one PSUM->SBUF copy at the end.
  * gc|gg from a small f16 side input (one matmul pair, mid-stream).
  * Output: a prepare_only dma_scatter_add writes descriptors during the
    stream; a Pool trigger_dma fires them once the [G,4,T] staging tile is
    ready -- skipping the whole HWDGE SEQ/gen/delay chain at the tail.
  * Host does only the [3, G, T]-level nonlinear tail.
"""

import sys

import numpy as np

try:
    import concourse  # noqa: F401
except ImportError:
    sys.path.insert(0, "/opt/trn_rl_repo")

import ml_dtypes

import concourse.tile as tile
from concourse import bacc, mybir
from concourse.bass_utils import run_bass_kernel_spmd

B, N, T, G = 4, 512, 24, 32
TOL, EPS = 0.05, 1e-6
P = 128                 # SBUF partitions
NLOC = N // 2           # rows per core (n-half)
A = 8                   # n-rows per block (partition sub-dim)
C = 16                  # m-blocks on partitions (partition sub-dim)
Q = N // C              # m-columns per msub block (free dim)
QP = Q // 2             # q-pairs for the col-term matmuls
NBLK = NLOC // A        # 32 blocks of 8 n-rows
PAIRS = NBLK // 2       # DoubleRow pairs
F = Q * T               # free elements per block
DMA_BLOCKS = ((0, 5), (5, 11), (11, 17), (17, 23), (23, 28),
              (28, 30), (30, 32))   # stream DMA block ranges
SMW = G + 2 * T         # smt row: [mt | cons | gen] per n
BLOB_ROWL = NBLK * G                   # 1024 u8 per partition
BLOB_SMT = 2 * SMW * 2                 # 320 u8 (f16) per partition
BLOB_COLN = QP * 2 * G                 # 1024 u8 per partition
BLOBW = BLOB_ROWL + BLOB_SMT + BLOB_COLN
OW = 3 * T              # out row: [gc | gg | net] per group

_F32 = mybir.dt.float32
_F16 = mybir.dt.float16
_F8 = mybir.dt.float8e4
_U8 = mybir.dt.uint8
_I16 = mybir.dt.int16


def _build_program():
    nc = bacc.Bacc("TRN2", target_bir_lowering=False, debug=False,
                   enable_asserts=False, num_devices=8)
    s = nc.dram_tensor("s", [NLOC, N, T], _F32, kind="ExternalInput").ap()
    blob = nc.dram_tensor("blob", [P, BLOBW], _U8, kind="ExternalInput").ap()
    # kv_writeback layout: [batch, d_head_inner, d_head_outer, n_ctx]
    out = nc.dram_tensor("out", [1, P, 1, OW], _F32, kind="ExternalOutput").ap()

    with tile.TileContext(nc) as tc, nc.allow_low_precision(
            "fp8 S stream + fp8 {0,1} masks, f32 PSUM accumulation"):
        with (
            tc.tile_pool(name="sb", bufs=1) as sb,
            tc.tile_pool(name="ps", bufs=1, space="PSUM") as ps,
        ):
            blobt = sb.tile([P, BLOBW], _U8, tag="blobt")
            stile = sb.tile([P, NBLK, F], _F8, tag="stile")
            # writeback staging: partition p carries [gc_p | gg_p | net_p] as
            # 72 contiguous f32 (partitions 32..127 are zeroed junk the host
            # ignores); kv_writeback streams the whole [128, 72] block out.
            src4 = sb.tile([P, 1, 1, OW], _F32, tag="src")
            src = src4[:, 0, 0]
            ctxi = sb.tile([P, 1], mybir.dt.int32, tag="ctxi")

            # --- all small inputs in ONE SP HWDGE DMA, hoisted pre-barrier
            # by the post-compile surgery so its transfer (848ns) runs in the
            # SWDGE warm-up window and the stream follows seamlessly ---
            nc.sync.dma_start(out=blobt, in_=blob)

            rowLv = blobt[:, 0:BLOB_ROWL].bitcast(_F8).rearrange(
                "p (b g) -> p b g", b=NBLK)
            smtv = blobt[:, BLOB_ROWL:BLOB_ROWL + BLOB_SMT].bitcast(
                _F16).rearrange("p (nb f) -> p nb f", nb=2)
            colNv = blobt[:, BLOB_ROWL + BLOB_SMT:BLOBW].bitcast(
                _F8).rearrange("p (j k g) -> p j k g", j=QP, k=2)

            # three banks, all at partition base 0
            gcp = ps.tile([G, T], _F32, tag="gcp")
            ggp = ps.tile([G, T], _F32, tag="ggp")
            netp = ps.tile([G, T], _F32, tag="netp")
            nc.vector.memset(src[:], 0.0)
            nc.vector.memset(ctxi[:], 0)

            # --- S stream: fp8 cast DMAs on the SWDGE ring ---
            # partition p = na*C + msub; block blk: n = blk*A + na;
            # free = (q, t) with m = msub*Q + q.
            s_r = s.rearrange("(blk a) (c q) t -> (a c) blk (q t)", a=A, c=C)
            for b0, b1 in DMA_BLOCKS:
                nc.gpsimd.dma_start(
                    out=stile[:, b0:b1, :].rearrange("p b f -> p (b f)"),
                    in_=s_r[:, b0:b1, :])

            # --- output path: descriptors prepared during the stream, the
            # trigger fires them once `src` is fully written ---
            dma_sem = nc.alloc_semaphore("outdma")
            nc.gpsimd.kv_writeback(
                out, src4[:], ctxi[:], prepare_only=True, sem=dma_sem)

            # --- PE pass ---
            # gc|gg projections first in PE program order: smt arrives with
            # the blob (~2us), well before the first S pair is consumable.
            for nb in range(2):
                nc.tensor.matmul(gcp, smtv[:, nb, 0:G],
                                 smtv[:, nb, G:G + T],
                                 start=(nb == 0), stop=(nb == 1),
                                 skip_group_check=True)
                nc.tensor.matmul(ggp, smtv[:, nb, 0:G],
                                 smtv[:, nb, G + T:],
                                 start=(nb == 0), stop=(nb == 1),
                                 skip_group_check=True)

            # One [G, T] accumulation group over all 1024 DoubleRow matmuls:
            # row term adds M[g,n]-projected q-slices (2 blocks per pass),
            # col term adds -M[g,m]-weighted q-pairs (2 q per pass).
            netw = netp
            for pr in range(PAIRS):
                lhs_row = rowLv[:, 2 * pr:2 * pr + 2, :]
                for q in range(Q):
                    nc.tensor.matmul(
                        netw, lhs_row,
                        stile[:, 2 * pr:2 * pr + 2, q * T:(q + 1) * T],
                        start=(pr == 0 and q == 0), stop=False,
                        perf_mode=mybir.MatmulPerfMode.DoubleRow,
                        skip_group_check=True)
                for blk in (2 * pr, 2 * pr + 1):
                    for j in range(QP):
                        nc.tensor.matmul(
                            netw, colNv[:, j],
                            stile[:, blk, 2 * j * T:(2 * j + 2) * T]
                            .rearrange("p (k t) -> p k t", k=2),
                            start=False,
                            stop=(pr == PAIRS - 1 and blk == 2 * pr + 1
                                  and j == QP - 1),
                            perf_mode=mybir.MatmulPerfMode.DoubleRow,
                            skip_group_check=True)

            # --- stage [gc | gg | net] and fire the scatter ---
            # Partition-preserving ACT copies; only the net copy is on the
            # post-stream critical path.
            act_done = nc.alloc_semaphore("actdone")
            nc.scalar.copy(out=src[0:G, 0:T], in_=gcp)
            nc.scalar.copy(out=src[0:G, T:2 * T], in_=ggp)
            nc.scalar.copy(out=src[0:G, 2 * T:3 * T], in_=netp)
            # Placeholder gate (>=0 so the schedule-time sim sails through);
            # post-compile surgery points it at the tile Activation engine sem
            # (ACT instructions cannot carry a second sync update, and the
            # trigger cannot carry a second wait).
            nc.gpsimd.wait_ge(act_done, 0)
            nc.gpsimd.trigger_dma(count=None)
    nc.compile()
    # Drop the framework's const-tensor memsets: nothing reads them, but they
    # run on the Pool engine ahead of the barrier and delay the first SWDGE
    # descriptor emission of the S stream.
    for blk in nc.m.functions[0].blocks:
        blk.instructions = [
            i for i in blk.instructions
            if not (type(i).__name__ == "InstMemset"
                    and i.outs and "const-" in str(i.outs[0]))
        ]
    # Tile schedules the scatter prep on a DMASW lane and the exit drain
    # waits on that lane's sem, but the descriptor-baked completion sem
    # (on_update[0], hardware increments by 16) is the user sem= kwarg.
    # Point on_update[0] at the orphaned DMASW sem so the DMA engines bump
    # the sem the drain actually waits on.
    fn = nc.m.functions[0]
    updated, waited, prep = set(), {}, None
    for blk in fn.blocks:
        for ins in blk.instructions:
            if type(ins).__name__ == "InstKVWritebackAnt":
                prep = ins
            si = ins.sync_info
            if si is None:
                continue
            for u in si.on_update:
                updated.add(u.id)
            for w in si.on_wait:
                waited[w.id] = w
    orphans = [w for wid, w in waited.items()
               if wid not in updated and (w.ant_name or "").startswith("DMASW")]
    assert prep is not None and len(orphans) == 1, (prep, orphans)
    u0 = prep.sync_info.on_update[0]
    assert u0.ant_name == "outdma", u0
    prep.sync_info.on_update[0] = mybir.SyncUpdate(
        sync_type=u0.sync_type, id=orphans[0].id, ant_name=orphans[0].ant_name,
        update_mode=u0.update_mode, update_value=u0.update_value,
        update_reg=u0.update_reg)
    # The sem-assignment pass drops the trigger's cross-engine RAW waits (it
    # only gates on the prep's Pool tick), so the trigger could fire before
    # the staging copies.  The placeholder wait_ge(actdone) sits right before
    # the trigger on the Pool SEQ; point it at the Activation engine-proc sem
    # with the cumulative tick of the last staging copy.
    # The scheduler can linearize the ACT exit drain (which waits on the
    # writeback's DMASW sem) BEFORE the staging copies on the same engine --
    # circular in strict block order.  Move the copies ahead of any
    # instruction waiting on the orphan sem.
    orphan_id = orphans[0].id
    for blk in fn.blocks:
        insts = blk.instructions
        drain_pos = None
        for i, ins in enumerate(insts):
            si = ins.sync_info
            if si and any(w.id == orphan_id for w in si.on_wait):
                drain_pos = i
                break
        if drain_pos is None:
            continue
        late = [ins for ins in insts[drain_pos:]
                if type(ins).__name__ == "InstActivation"]
        if late:
            rest = [ins for ins in insts if ins not in late]
            blk.instructions = (rest[:drain_pos] + late + rest[drain_pos:])
    # The framework's ACT table load lands in the postamble AFTER the exit
    # wait on the writeback sem, adding ~1.3us of pure tail.  Hoist it to the
    # head of the main block so it overlaps the stream (baseline behavior).
    loads = []
    for blk in fn.blocks:
        keep = []
        for ins in blk.instructions:
            if type(ins).__name__ == "InstLoadActFuncSet":
                loads.append(ins)
            else:
                keep.append(ins)
        blk.instructions = keep
    if loads:
        main = fn.blocks[1]
        main.instructions = loads + main.instructions
    # The trigger can carry only one codegen sync wait; point it at the ACT
    # engine sem tick of the last staging copy (the prep's descriptor gen on
    # the Pool engine finishes several microseconds earlier, so dropping the
    # Pool tick wait is safe).  Delete the placeholder gate entirely.
    act_total = 0
    last_src_tick = None
    trig = None
    gate = None
    for blk in fn.blocks:
        for ins in blk.instructions:
            if type(ins).__name__ == "InstTriggerDma":
                trig = ins
            si = ins.sync_info
            if si is None:
                continue
            for w in si.on_wait:
                if w.ant_name == "actdone":
                    gate = ins
            for u in si.on_update:
                if (u.ant_name or "").startswith("Activation_"):
                    act_total += (u.update_value or 1)
                    if type(ins).__name__ == "InstActivation":
                        last_src_tick = (u.id, u.ant_name, act_total)
    assert trig is not None and last_src_tick is not None, (trig, last_src_tick)
    sid, sname, val = last_src_tick
    trig.sync_info.on_wait = [mybir.SyncWait(
        sync_type="semaphore", id=sid, ant_name=sname,
        wait_mode="sem-ge-imm", wait_value=val, wait_reg=None)]
    if gate is not None:
        for blk in fn.blocks:
            blk.instructions = [i for i in blk.instructions if i is not gate]
    # Hoist the blob HWDGE DMA (SP) and the first stream DMA (Pool) ahead of
    # the entry barrier: their descriptors have no dependencies, so the first
    # transfer starts ~1.3us in instead of ~2.2us.
    main = fn.blocks[1]
    hoist = []
    seen_pool = seen_sp = False
    keep = []
    for ins in main.instructions:
        if (type(ins).__name__ == "InstDMACopy" and not seen_sp
                and ins.engine == mybir.EngineType.SP):
            hoist.append(ins)
            seen_sp = True
        elif (type(ins).__name__ == "InstDMACopy" and not seen_pool
                and ins.engine == mybir.EngineType.Pool):
            hoist.append(ins)
            seen_pool = True
        else:
            keep.append(ins)
    main.instructions = keep
    fn.blocks[0].instructions = hoist + fn.blocks[0].instructions
    return nc


_NC_CACHE = None


def _get_program():
    global _NC_CACHE
    if _NC_CACHE is None:
        _NC_CACHE = _build_program()
    return _NC_CACHE


_RUNNER_CACHE = None


def _get_runner():
    """Compiled-once jit(shard_map) executor over 8 cores."""
    global _RUNNER_CACHE
    if _RUNNER_CACHE is None:
        import jax
        from jax.sharding import Mesh, PartitionSpec
        from jax.experimental.shard_map import shard_map
        from concourse import bass2jax, mybir as mb

        nc = _get_program()
        bass2jax.install_neuronx_cc_hook()
        partition_name = (nc.partition_id_tensor.name
                          if nc.partition_id_tensor else None)
        in_names, out_names, out_avals = [], [], []
        for alloc in nc.m.functions[0].allocations:
            if not isinstance(alloc, mb.MemoryLocationSet):
                continue
            name = alloc.memorylocations[0].name
            if alloc.kind == "ExternalInput":
                if name != partition_name:
                    in_names.append(name)
            elif alloc.kind == "ExternalOutput":
                out_names.append(name)
                out_avals.append(jax.core.ShapedArray(
                    tuple(alloc.tensor_shape), mb.dt.np(alloc.dtype)))
        n_params = len(in_names)
        all_names = in_names + out_names
        if partition_name is not None:
            all_names = all_names + [partition_name]

        def _body(*args):
            operands = list(args)
            if partition_name is not None:
                operands.append(bass2jax.partition_id_tensor())
            outs = bass2jax._bass_exec_p.bind(
                *operands,
                out_avals=tuple(out_avals),
                in_names=tuple(all_names),
                out_names=tuple(out_names),
                lowering_input_output_aliases=(),
                sim_require_finite=True,
                sim_require_nnan=True,
                nc=nc,
            )
            return tuple(outs)

        devices = jax.devices()[:8]
        mesh = Mesh(np.asarray(devices), ("core",))
        n_outs = len(out_names)
        sharded = jax.jit(
            shard_map(_body, mesh=mesh,
                      in_specs=(PartitionSpec("core"),) * (n_params + n_outs),
                      out_specs=(PartitionSpec("core"),) * n_outs,
                      check_rep=False),
            donate_argnums=tuple(range(n_params, n_params + n_outs)),
            keep_unused=True,
        )
        _RUNNER_CACHE = (sharded, in_names[:n_params], out_names, out_avals)
    return _RUNNER_CACHE


def _host_side(consumption, generation, sharing_matrix, lv_group_ids,
               valid_lv_mask):
    """Shared input prep: per-core input maps."""
    consumption = np.ascontiguousarray(consumption, dtype=np.float32)
    generation = np.ascontiguousarray(generation, dtype=np.float32)
    sharing_matrix = np.ascontiguousarray(sharing_matrix, dtype=np.float32)
    ids = np.asarray(lv_group_ids)
    valid = np.asarray(valid_lv_mask, dtype=np.float32)

    onehot = (ids[None, :] == np.arange(G)[:, None]).astype(np.float32)
    n_unique = np.float32(np.unique(ids).size)
    M = onehot * valid[None, :]                      # [G, N]
    mt = np.ascontiguousarray(M.T)                   # [N, G]

    # coln[p=(na,msub), j, k, g] = -M[g, msub*Q + 2j + k]  (negated imports)
    cw = (-mt).reshape(C, Q, G)                      # [msub, q, g]
    coln = np.broadcast_to(cw[None], (A, C, Q, G)).reshape(P, Q * G)
    coln8 = np.ascontiguousarray(
        coln.astype(ml_dtypes.float8_e4m3).view(np.uint8))

    in_maps = []
    for c in range(8):
        b, hh = divmod(c, 2)
        sl = slice(hh * NLOC, (hh + 1) * NLOC)
        mt_half = mt[sl]                             # [NLOC, G]
        # rowL[p=(na,msub), blk, g] = M[g, blk*A + na]
        proj = mt_half.reshape(NBLK, A, G).transpose(1, 0, 2)   # [na, blk, g]
        proj = np.broadcast_to(proj[:, None], (A, C, NBLK, G))
        rowl8 = proj.reshape(P, NBLK * G).astype(
            ml_dtypes.float8_e4m3).view(np.uint8)
        # smt[p, nb, f]: f = [mt row | cons row | gen row] for n = nb*P + p
        sm = np.empty((2, P, SMW), np.float16)
        sm[:, :, :G] = mt_half.reshape(2, P, G)
        sm[:, :, G:G + T] = consumption[b, sl].reshape(2, P, T)
        sm[:, :, G + T:] = generation[b, sl].reshape(2, P, T)
        sm_bytes = sm.transpose(1, 0, 2).reshape(P, -1).view(np.uint8)

        blob = np.zeros((P, BLOBW), np.uint8)
        blob[:, :BLOB_ROWL] = rowl8
        blob[:, BLOB_ROWL:BLOB_ROWL + BLOB_SMT] = sm_bytes
        blob[:, BLOB_ROWL + BLOB_SMT:] = coln8
        in_maps.append({
            "s": np.ascontiguousarray(sharing_matrix[b, sl]),
            "blob": np.ascontiguousarray(blob),
        })
    return in_maps, n_unique


def kernel(consumption, generation, sharing_matrix, lv_group_ids,
           valid_lv_mask, imbalance_penalty_weight, _want_results=False,
           **run_kwargs):
    w = np.float32(np.asarray(imbalance_penalty_weight))
    in_maps, n_unique = _host_side(consumption, generation, sharing_matrix,
                                   lv_group_ids, valid_lv_mask)
    res = None
    if _want_results or run_kwargs:
        nc = _get_program()
        res = run_bass_kernel_spmd(nc, in_maps, core_ids=list(range(8)),
                                   **run_kwargs)
        parts = np.stack([res.results[c]["out"] for c in range(8)])
    else:
        try:
            fn, in_names, out_names, out_avals = _get_runner()
            concat_in = [np.concatenate([m[name] for m in in_maps], axis=0)
                         for name in in_names]
            zeros = [np.zeros((8 * a.shape[0], *a.shape[1:]), a.dtype)
                     for a in out_avals]
            out_arrs = fn(*concat_in, *zeros)
            parts = np.asarray(out_arrs[out_names.index("out")]).reshape(
                8, P, OW)
        except Exception:
            nc = _get_program()
            res = run_bass_kernel_spmd(nc, in_maps, core_ids=list(range(8)))
            parts = np.stack([res.results[c]["out"] for c in range(8)]).reshape(
                8, P, OW)
    # partition p (< G) carries [gc_p | gg_p | net_p] as 3*T columns
    per_core = parts[:, :G, :].reshape(8, G, 3, T).transpose(0, 2, 1, 3)
    full = per_core.reshape(B, 2, 3, G, T).sum(axis=1, dtype=np.float32)
    gc, gg, net = full[:, 0], full[:, 1], full[:, 2]

    imbalance = np.abs(gc - gg + net)
    total = gc + gg + np.float32(EPS)
    pen = np.maximum(imbalance / total - np.float32(TOL), np.float32(0))
    outv = np.float32(pen.sum(dtype=np.float32) * w / n_unique)
    out_arr = np.array(outv, dtype=np.float32)
    if _want_results:
        return out_arr, res
    return out_arr


# revision 53
# speedup vs baseline: 1.2140x; 1.0299x over previous
"""Trainium2 Bass kernel for nn_EnergyBalanceChecker (segment_reduce), v5.

Problem (hardcoded): B=4, N=512, T=24, G=32, TOL=0.05, EPS=1e-6.

  M = onehot(lv_group_ids) * valid_lv_mask                     # [G, N]
  gc  = einsum('gn,bnt->bgt', M, consumption)
  gg  = einsum('gn,bnt->bgt', M, generation)
  net = einsum('gn,bnt->bgt', M, S.sum(axis=2) - S.sum(axis=1))
  pen = relu(|gc-gg+net| / (gc+gg+eps) - TOL);  out = pen.sum()*w/n_unique

Sharding: 8 cores = 4 batches x 2 halves of the (row) N axis.

v5 dataflow (vs v4): the q-axis fold moves INTO the matmul pass, so there
is no wide-PSUM drain at all, and the output leaves via a pre-prepared
SWDGE scatter fired by a trigger instruction:
  * S streams in fp8e4 as before (SWDGE cast DMAs; cost is charged on
    destination bytes).  Partitions carry (na in 8 n-rows) x (msub in 16
    m-blocks); free = (q, t).
  * Row term: per (block-pair, q) a DoubleRow matmul with the M[g,n]
    projection lhsT and a T-wide rhs slice accumulates straight into a
    single [G, T] PSUM tile (q and pairs both fold in PSUM).
  * Col term: per (block, q-pair) a DoubleRow matmul pairs two q slices
    of one block with per-q lhsT columns -M[g, m(msub,q)] -- accumulating
    *negated* imports into the SAME [G, T] tile, so net = row - col needs
    no subtract, just one PSUM->SBUF copy at the end.
  * gc|gg from a small f16 side input (one matmul pair, mid-stream).
  * Output: a prepare_only dma_scatter_add writes descriptors during the
    stream; a Pool trigger_dma fires them once the [G,4,T] staging tile is
    ready -- skipping the whole HWDGE SEQ/gen/delay chain at the tail.
  * Host does only the [3, G, T]-level nonlinear tail.
"""

import sys

import numpy as np

try:
    import concourse  # noqa: F401
except ImportError:
    sys.path.insert(0, "/opt/trn_rl_repo")

import ml_dtypes

import concourse.tile as tile
from concourse import bacc, mybir
from concourse.bass_utils import run_bass_kernel_spmd

B, N, T, G = 4, 512, 24, 32
TOL, EPS = 0.05, 1e-6
P = 128                 # SBUF partitions
NLOC = N // 2           # rows per core (n-half)
A = 8                   # n-rows per block (partition sub-dim)
C = 16                  # m-blocks on partitions (partition sub-dim)
Q = N // C              # m-columns per msub block (free dim)
QP = Q // 2             # q-pairs for the col-term matmuls
NBLK = NLOC // A        # 32 blocks of 8 n-rows
PAIRS = NBLK // 2       # DoubleRow pairs
F = Q * T               # free elements per block
DMA_BLOCKS = ((0, 5), (5, 11), (11, 17), (17, 23), (23, 28),
              (28, 30), (30, 32))   # stream DMA block ranges
SMW = G + 2 * T         # smt row: [mt | cons | gen] per n
BLOB_ROWL = NBLK * G                   # 1024 u8 per partition
BLOB_SMT = 2 * SMW * 2                 # 320 u8 (f16) per partition
BLOB_COLN = QP * 2 * G                 # 1024 u8 per partition
BLOBW = BLOB_ROWL + BLOB_SMT + BLOB_COLN
OW = 3 * T              # out row: [gc | gg | net] per group

_F32 = mybir.dt.float32
_F16 = mybir.dt.float16
_F8 = mybir.dt.float8e4
_U8 = mybir.dt.uint8
_I16 = mybir.dt.int16


def _build_program():
    nc = bacc.Bacc("TRN2", target_bir_lowering=False, debug=False,
                   enable_asserts=False, num_devices=8)
    s = nc.dram_tensor("s", [NLOC, N, T], _F32, kind="ExternalInput").ap()
    blob = nc.dram_tensor("blob", [P, BLOBW], _U8, kind="ExternalInput").ap()
    # kv_writeback layout: [batch, d_head_inner, d_head_outer, n_ctx]
    out = nc.dram_tensor("out", [1, P, 1, OW], _F32, kind="ExternalOutput").ap()

    with tile.TileContext(nc) as tc, nc.allow_low_precision(
            "fp8 S stream + fp8 {0,1} masks, f32 PSUM accumulation"):
        with (
            tc.tile_pool(name="sb", bufs=1) as sb,
            tc.tile_pool(name="ps", bufs=1, space="PSUM") as ps,
        ):
            blobt = sb.tile([P, BLOBW], _U8, tag="blobt")
            stile = sb.tile([P, NBLK, F], _F8, tag="stile")
            # writeback staging: partition p carries [gc_p | gg_p | net_p] as
            # 72 contiguous f32 (partitions 32..127 are zeroed junk the host
            # ignores); kv_writeback streams the whole [128, 72] block out.
            src4 = sb.tile([P, 1, 1, OW], _F32, tag="src")
            src = src4[:, 0, 0]
            ctxi = sb.tile([P, 1], mybir.dt.int32, tag="ctxi")

            # --- all small inputs in ONE SP HWDGE DMA, hoisted pre-barrier
            # by the post-compile surgery so its transfer (848ns) runs in the
            # SWDGE warm-up window and the stream follows seamlessly ---
            nc.sync.dma_start(out=blobt, in_=blob)

            rowLv = blobt[:, 0:BLOB_ROWL].bitcast(_F8).rearrange(
                "p (b g) -> p b g", b=NBLK)
            smtv = blobt[:, BLOB_ROWL:BLOB_ROWL + BLOB_SMT].bitcast(
                _F16).rearrange("p (nb f) -> p nb f", nb=2)
            colNv = blobt[:, BLOB_ROWL + BLOB_SMT:BLOBW].bitcast(
                _F8).rearrange("p (j k g) -> p j k g", j=QP, k=2)

            # three banks, all at partition base 0
            gcp = ps.tile([G, T], _F32, tag="gcp")
            ggp = ps.tile([G, T], _F32, tag="ggp")
            netp = ps.tile([G, T], _F32, tag="netp")
            nc.vector.memset(src[:], 0.0)
            nc.vector.memset(ctxi[:], 0)

            # --- S stream: fp8 cast DMAs on the SWDGE ring ---
            # partition p = na*C + msub; block blk: n = blk*A + na;
            # free = (q, t) with m = msub*Q + q.
            s_r = s.rearrange("(blk a) (c q) t -> (a c) blk (q t)", a=A, c=C)
            for b0, b1 in DMA_BLOCKS:
                nc.gpsimd.dma_start(
                    out=stile[:, b0:b1, :].rearrange("p b f -> p (b f)"),
                    in_=s_r[:, b0:b1, :])

            # --- output path: descriptors prepared during the stream, the
            # trigger fires them once `src` is fully written ---
            dma_sem = nc.alloc_semaphore("outdma")
            nc.gpsimd.kv_writeback(
                out, src4[:], ctxi[:], prepare_only=True, sem=dma_sem)

            # --- PE pass ---
            # gc|gg projections first in PE program order: smt arrives with
            # the blob (~2us), well before the first S pair is consumable.
            for nb in range(2):
                nc.tensor.matmul(gcp, smtv[:, nb, 0:G],
                                 smtv[:, nb, G:G + T],
                                 start=(nb == 0), stop=(nb == 1),
                                 skip_group_check=True)
                nc.tensor.matmul(ggp, smtv[:, nb, 0:G],
                                 smtv[:, nb, G + T:],
                                 start=(nb == 0), stop=(nb == 1),
                                 skip_group_check=True)

            # One [G, T] accumulation group over all 1024 DoubleRow matmuls:
            # row term adds M[g,n]-projected q-slices (2 blocks per pass),
            # col term adds -M[g,m]-weighted q-pairs (2 q per pass).
            netw = netp
            for pr in range(PAIRS):
                lhs_row = rowLv[:, 2 * pr:2 * pr + 2, :]
                for q in range(Q):
                    nc.tensor.matmul(
                        netw, lhs_row,
                        stile[:, 2 * pr:2 * pr + 2, q * T:(q + 1) * T],
                        start=(pr == 0 and q == 0), stop=False,
                        perf_mode=mybir.MatmulPerfMode.DoubleRow,
                        skip_group_check=True)
                for blk in (2 * pr, 2 * pr + 1):
                    for j in range(QP):
                        nc.tensor.matmul(
                            netw, colNv[:, j],
                            stile[:, blk, 2 * j * T:(2 * j + 2) * T]
                            .rearrange("p (k t) -> p k t", k=2),
                            start=False,
                            stop=(pr == PAIRS - 1 and blk == 2 * pr + 1
                                  and j == QP - 1),
                            perf_mode=mybir.MatmulPerfMode.DoubleRow,
                            skip_group_check=True)

            # --- stage [gc | gg | net] and fire the scatter ---
            # Partition-preserving ACT copies; only the net copy is on the
            # post-stream critical path.
            act_done = nc.alloc_semaphore("actdone")
            nc.scalar.copy(out=src[0:G, 0:T], in_=gcp)
            nc.scalar.copy(out=src[0:G, T:2 * T], in_=ggp)
            nc.scalar.copy(out=src[0:G, 2 * T:3 * T], in_=netp)
            # Placeholder gate (>=0 so the schedule-time sim sails through);
            # post-compile surgery points it at the tile Activation engine sem
            # (ACT instructions cannot carry a second sync update, and the
            # trigger cannot carry a second wait).
            nc.gpsimd.wait_ge(act_done, 0)
            nc.gpsimd.trigger_dma(count=None)
    nc.compile()
    # Drop the framework's const-tensor memsets: nothing reads them, but they
    # run on the Pool engine ahead of the barrier and delay the first SWDGE
    # descriptor emission of the S stream.
    for blk in nc.m.functions[0].blocks:
        blk.instructions = [
            i for i in blk.instructions
            if not (type(i).__name__ == "InstMemset"
                    and i.outs and "const-" in str(i.outs[0]))
        ]
    # Tile schedules the scatter prep on a DMASW lane and the exit drain
    # waits on that lane's sem, but the descriptor-baked completion sem
    # (on_update[0], hardware increments by 16) is the user sem= kwarg.
    # Point on_update[0] at the orphaned DMASW sem so the DMA engines bump
    # the sem the drain actually waits on.
    fn = nc.m.functions[0]
    updated, waited, prep = set(), {}, None
    for blk in fn.blocks:
        for ins in blk.instructions:
            if type(ins).__name__ == "InstKVWritebackAnt":
                prep = ins
            si = ins.sync_info
            if si is None:
                continue
            for u in si.on_update:
                updated.add(u.id)
            for w in si.on_wait:
                waited[w.id] = w
    orphans = [w for wid, w in waited.items()
               if wid not in updated and (w.ant_name or "").startswith("DMASW")]
    assert prep is not None and len(orphans) == 1, (prep, orphans)
    u0 = prep.sync_info.on_update[0]
    assert u0.ant_name == "outdma", u0
    prep.sync_info.on_update[0] = mybir.SyncUpdate(
        sync_type=u0.sync_type, id=orphans[0].id, ant_name=orphans[0].ant_name,
        update_mode=u0.update_mode, update_value=u0.update_value,
        update_reg=u0.update_reg)
    # The sem-assignment pass drops the trigger's cross-engine RAW waits (it
    # only gates on the prep's Pool tick), so the trigger could fire before
    # the staging copies.  The placeholder wait_ge(actdone) sits right before
    # the trigger on the Pool SEQ; point it at the Activation engine-proc sem
    # with the cumulative tick of the last staging copy.
    # The scheduler can linearize the ACT exit drain (which waits on the
    # writeback's DMASW sem) BEFORE the staging copies on the same engine --
    # circular in strict block order.  Move the copies ahead of any
    # instruction waiting on the orphan sem.
    orphan_id = orphans[0].id
    for blk in fn.blocks:
        insts = blk.instructions
        drain_pos = None
        for i, ins in enumerate(insts):
            si = ins.sync_info
            if si and any(w.id == orphan_id for w in si.on_wait):
                drain_pos = i
                break
        if drain_pos is None:
            continue
        late = [ins for ins in insts[drain_pos:]
                if type(ins).__name__ == "InstActivation"]
        if late:
            rest = [ins for ins in insts if ins not in late]
            blk.instructions = (rest[:drain_pos] + late + rest[drain_pos:])
    # The framework's ACT table load lands in the postamble AFTER the exit
    # wait on the writeback sem, adding ~1.3us of pure tail.  Hoist it to the
    # head of the main block so it overlaps the stream (baseline behavior).
    loads = []
    for blk in fn.blocks:
        keep = []
        for ins in blk.instructions:
            if type(ins).__name__ == "InstLoadActFuncSet":
                loads.append(ins)
            else:
                keep.append(ins)
        blk.instructions = keep
    if loads:
        main = fn.blocks[1]
        main.instructions = loads + main.instructions
    # The trigger can carry only one codegen sync wait; point it at the ACT
    # engine sem tick of the last staging copy (the prep's descriptor gen on
    # the Pool engine finishes several microseconds earlier, so dropping the
    # Pool tick wait is safe).  Delete the placeholder gate entirely.
    act_total = 0
    last_src_tick = None
    trig = None
    gate = None
    for blk in fn.blocks:
        for ins in blk.instructions:
            if type(ins).__name__ == "InstTriggerDma":
                trig = ins
            si = ins.sync_info
            if si is None:
                continue
            for w in si.on_wait:
                if w.ant_name == "actdone":
                    gate = ins
            for u in si.on_update:
                if (u.ant_name or "").startswith("Activation_"):
                    act_total += (u.update_value or 1)
                    if type(ins).__name__ == "InstActivation":
                        last_src_tick = (u.id, u.ant_name, act_total)
    assert trig is not None and last_src_tick is not None, (trig, last_src_tick)
    sid, sname, val = last_src_tick
    trig.sync_info.on_wait = [mybir.SyncWait(
        sync_type="semaphore", id=sid, ant_name=sname,
        wait_mode="sem-ge-imm", wait_value=val, wait_reg=None)]
    if gate is not None:
        for blk in fn.blocks:
            blk.instructions = [i for i in blk.instructions if i is not gate]
    # Hoist the blob HWDGE DMA (SP) and the first stream DMA (Pool) ahead of
    # the entry barrier: their descriptors have no dependencies, so the first
    # transfer starts ~1.3us in instead of ~2.2us.
    main = fn.blocks[1]
    hoist = []
    seen_pool = seen_sp = False
    keep = []
    for ins in main.instructions:
        if (type(ins).__name__ == "InstDMACopy" and not seen_sp
                and ins.engine == mybir.EngineType.SP):
            hoist.append(ins)
            seen_sp = True
        elif (type(ins).__name__ == "InstDMACopy" and not seen_pool
                and ins.engine == mybir.EngineType.Pool):
            hoist.append(ins)
            seen_pool = True
        else:
            keep.append(ins)
    main.instructions = keep
    fn.blocks[0].instructions = hoist + fn.blocks[0].instructions
    # Exit-barrier trim: the writeback completion is already enforced by the
    # per-engine DMASW waits in the exit block; the trailing gather/release
    # barrier rounds only synchronize engine end times.  Drop them so the
    # kernel ends when the last DMASW waiter releases.
    exit_blk = fn.blocks[-1]
    exit_blk.instructions = [
        i for i in exit_blk.instructions
        if not i.name.startswith("barrier_")
    ]
    return nc


_NC_CACHE = None


def _get_program():
    global _NC_CACHE
    if _NC_CACHE is None:
        _NC_CACHE = _build_program()
    return _NC_CACHE


_RUNNER_CACHE = None


def _get_runner():
    """Compiled-once jit(shard_map) executor over 8 cores."""
    global _RUNNER_CACHE
    if _RUNNER_CACHE is None:
        import jax
        from jax.sharding import Mesh, PartitionSpec
        from jax.experimental.shard_map import shard_map
        from concourse import bass2jax, mybir as mb

        nc = _get_program()
        bass2jax.install_neuronx_cc_hook()
        partition_name = (nc.partition_id_tensor.name
                          if nc.partition_id_tensor else None)
        in_names, out_names, out_avals = [], [], []
        for alloc in nc.m.functions[0].allocations:
            if not isinstance(alloc, mb.MemoryLocationSet):
                continue
            name = alloc.memorylocations[0].name
            if alloc.kind == "ExternalInput":
                if name != partition_name:
                    in_names.append(name)
            elif alloc.kind == "ExternalOutput":
                out_names.append(name)
                out_avals.append(jax.core.ShapedArray(
                    tuple(alloc.tensor_shape), mb.dt.np(alloc.dtype)))
        n_params = len(in_names)
        all_names = in_names + out_names
        if partition_name is not None:
            all_names = all_names + [partition_name]

        def _body(*args):
            operands = list(args)
            if partition_name is not None:
                operands.append(bass2jax.partition_id_tensor())
            outs = bass2jax._bass_exec_p.bind(
                *operands,
                out_avals=tuple(out_avals),
                in_names=tuple(all_names),
                out_names=tuple(out_names),
                lowering_input_output_aliases=(),
                sim_require_finite=True,
                sim_require_nnan=True,
                nc=nc,
            )
            return tuple(outs)

        devices = jax.devices()[:8]
        mesh = Mesh(np.asarray(devices), ("core",))
        n_outs = len(out_names)
        sharded = jax.jit(
            shard_map(_body, mesh=mesh,
                      in_specs=(PartitionSpec("core"),) * (n_params + n_outs),
                      out_specs=(PartitionSpec("core"),) * n_outs,
                      check_rep=False),
            donate_argnums=tuple(range(n_params, n_params + n_outs)),
            keep_unused=True,
        )
        _RUNNER_CACHE = (sharded, in_names[:n_params], out_names, out_avals)
    return _RUNNER_CACHE


def _host_side(consumption, generation, sharing_matrix, lv_group_ids,
               valid_lv_mask):
    """Shared input prep: per-core input maps."""
    consumption = np.ascontiguousarray(consumption, dtype=np.float32)
    generation = np.ascontiguousarray(generation, dtype=np.float32)
    sharing_matrix = np.ascontiguousarray(sharing_matrix, dtype=np.float32)
    ids = np.asarray(lv_group_ids)
    valid = np.asarray(valid_lv_mask, dtype=np.float32)

    onehot = (ids[None, :] == np.arange(G)[:, None]).astype(np.float32)
    n_unique = np.float32(np.unique(ids).size)
    M = onehot * valid[None, :]                      # [G, N]
    mt = np.ascontiguousarray(M.T)                   # [N, G]

    # coln[p=(na,msub), j, k, g] = -M[g, msub*Q + 2j + k]  (negated imports)
    cw = (-mt).reshape(C, Q, G)                      # [msub, q, g]
    coln = np.broadcast_to(cw[None], (A, C, Q, G)).reshape(P, Q * G)
    coln8 = np.ascontiguousarray(
        coln.astype(ml_dtypes.float8_e4m3).view(np.uint8))

    in_maps = []
    for c in range(8):
        b, hh = divmod(c, 2)
        sl = slice(hh * NLOC, (hh + 1) * NLOC)
        mt_half = mt[sl]                             # [NLOC, G]
        # rowL[p=(na,msub), blk, g] = M[g, blk*A + na]
        proj = mt_half.reshape(NBLK, A, G).transpose(1, 0, 2)   # [na, blk, g]
        proj = np.broadcast_to(proj[:, None], (A, C, NBLK, G))
        rowl8 = proj.reshape(P, NBLK * G).astype(
            ml_dtypes.float8_e4m3).view(np.uint8)
        # smt[p, nb, f]: f = [mt row | cons row | gen row] for n = nb*P + p
        sm = np.empty((2, P, SMW), np.float16)
        sm[:, :, :G] = mt_half.reshape(2, P, G)
        sm[:, :, G:G + T] = consumption[b, sl].reshape(2, P, T)
        sm[:, :, G + T:] = generation[b, sl].reshape(2, P, T)
        sm_bytes = sm.transpose(1, 0, 2).reshape(P, -1).view(np.uint8)

        blob = np.zeros((P, BLOBW), np.uint8)
        blob[:, :BLOB_ROWL] = rowl8
        blob[:, BLOB_ROWL:BLOB_ROWL + BLOB_SMT] = sm_bytes
        blob[:, BLOB_ROWL + BLOB_SMT:] = coln8
        in_maps.append({
            "s": np.ascontiguousarray(sharing_matrix[b, sl]),
            "blob": np.ascontiguousarray(blob),
        })
    return in_maps, n_unique


def kernel(consumption, generation, sharing_matrix, lv_group_ids,
           valid_lv_mask, imbalance_penalty_weight, _want_results=False,
           **run_kwargs):
    w = np.float32(np.asarray(imbalance_penalty_weight))
    in_maps, n_unique = _host_side(consumption, generation, sharing_matrix,
                                   lv_group_ids, valid_lv_mask)
    res = None
    if _want_results or run_kwargs:
        nc = _get_program()
        res = run_bass_kernel_spmd(nc, in_maps, core_ids=list(range(8)),
                                   **run_kwargs)
        parts = np.stack([res.results[c]["out"] for c in range(8)])
    else:
        try:
            fn, in_names, out_names, out_avals = _get_runner()
            concat_in = [np.concatenate([m[name] for m in in_maps], axis=0)
                         for name in in_names]
            zeros = [np.zeros((8 * a.shape[0], *a.shape[1:]), a.dtype)
                     for a in out_avals]
            out_arrs = fn(*concat_in, *zeros)
            parts = np.asarray(out_arrs[out_names.index("out")]).reshape(
                8, P, OW)
        except Exception:
            nc = _get_program()
            res = run_bass_kernel_spmd(nc, in_maps, core_ids=list(range(8)))
            parts = np.stack([res.results[c]["out"] for c in range(8)]).reshape(
                8, P, OW)
    # partition p (< G) carries [gc_p | gg_p | net_p] as 3*T columns
    per_core = parts[:, :G, :].reshape(8, G, 3, T).transpose(0, 2, 1, 3)
    full = per_core.reshape(B, 2, 3, G, T).sum(axis=1, dtype=np.float32)
    gc, gg, net = full[:, 0], full[:, 1], full[:, 2]

    imbalance = np.abs(gc - gg + net)
    total = gc + gg + np.float32(EPS)
    pen = np.maximum(imbalance / total - np.float32(TOL), np.float32(0))
    outv = np.float32(pen.sum(dtype=np.float32) * w / n_unique)
    out_arr = np.array(outv, dtype=np.float32)
    if _want_results:
        return out_arr, res
    return out_arr


# revision 56
# speedup vs baseline: 1.2197x; 1.0047x over previous
"""Trainium2 Bass kernel for nn_EnergyBalanceChecker (segment_reduce), v5.

Problem (hardcoded): B=4, N=512, T=24, G=32, TOL=0.05, EPS=1e-6.

  M = onehot(lv_group_ids) * valid_lv_mask                     # [G, N]
  gc  = einsum('gn,bnt->bgt', M, consumption)
  gg  = einsum('gn,bnt->bgt', M, generation)
  net = einsum('gn,bnt->bgt', M, S.sum(axis=2) - S.sum(axis=1))
  pen = relu(|gc-gg+net| / (gc+gg+eps) - TOL);  out = pen.sum()*w/n_unique

Sharding: 8 cores = 4 batches x 2 halves of the (row) N axis.

v5 dataflow (vs v4): the q-axis fold moves INTO the matmul pass, so there
is no wide-PSUM drain at all, and the output leaves via a pre-prepared
SWDGE scatter fired by a trigger instruction:
  * S streams in fp8e4 as before (SWDGE cast DMAs; cost is charged on
    destination bytes).  Partitions carry (na in 8 n-rows) x (msub in 16
    m-blocks); free = (q, t).
  * Row term: per (block-pair, q) a DoubleRow matmul with the M[g,n]
    projection lhsT and a T-wide rhs slice accumulates straight into a
    single [G, T] PSUM tile (q and pairs both fold in PSUM).
  * Col term: per (block, q-pair) a DoubleRow matmul pairs two q slices
    of one block with per-q lhsT columns -M[g, m(msub,q)] -- accumulating
    *negated* imports into the SAME [G, T] tile, so net = row - col needs
    no subtract, just one PSUM->SBUF copy at the end.
  * gc|gg from a small f16 side input (one matmul pair, mid-stream).
  * Output: a prepare_only dma_scatter_add writes descriptors during the
    stream; a Pool trigger_dma fires them once the [G,4,T] staging tile is
    ready -- skipping the whole HWDGE SEQ/gen/delay chain at the tail.
  * Host does only the [3, G, T]-level nonlinear tail.
"""

import sys

import numpy as np

try:
    import concourse  # noqa: F401
except ImportError:
    sys.path.insert(0, "/opt/trn_rl_repo")

import ml_dtypes

import concourse.tile as tile
from concourse import bacc, mybir
from concourse.bass_utils import run_bass_kernel_spmd

B, N, T, G = 4, 512, 24, 32
TOL, EPS = 0.05, 1e-6
P = 128                 # SBUF partitions
NLOC = N // 2           # rows per core (n-half)
A = 8                   # n-rows per block (partition sub-dim)
C = 16                  # m-blocks on partitions (partition sub-dim)
Q = N // C              # m-columns per msub block (free dim)
QP = Q // 2             # q-pairs for the col-term matmuls
NBLK = NLOC // A        # 32 blocks of 8 n-rows
PAIRS = NBLK // 2       # DoubleRow pairs
F = Q * T               # free elements per block
DMA_BLOCKS = ((0, 5), (5, 11), (11, 17), (17, 23), (23, 28),
              (28, 30), (30, 32))   # stream DMA block ranges
SMW = G + 2 * T         # smt row: [mt | cons | gen] per n
BLOB_ROWL = NBLK * G                   # 1024 u8 per partition
BLOB_SMT = 2 * SMW * 2                 # 320 u8 (f16) per partition
BLOB_COLN = QP * 2 * G                 # 1024 u8 per partition
BLOBW = BLOB_ROWL + BLOB_SMT + BLOB_COLN
OW = 3 * T              # out row: [gc | gg | net] per group

_F32 = mybir.dt.float32
_F16 = mybir.dt.float16
_F8 = mybir.dt.float8e4
_U8 = mybir.dt.uint8
_I16 = mybir.dt.int16


def _build_program():
    nc = bacc.Bacc("TRN2", target_bir_lowering=False, debug=False,
                   enable_asserts=False, num_devices=8)
    s = nc.dram_tensor("s", [NLOC, N, T], _F32, kind="ExternalInput").ap()
    blob = nc.dram_tensor("blob", [P, BLOBW], _U8, kind="ExternalInput").ap()
    # kv_writeback layout: [batch, d_head_inner, d_head_outer, n_ctx]
    out = nc.dram_tensor("out", [1, P, 1, OW], _F32, kind="ExternalOutput").ap()

    with tile.TileContext(nc) as tc, nc.allow_low_precision(
            "fp8 S stream + fp8 {0,1} masks, f32 PSUM accumulation"):
        with (
            tc.tile_pool(name="sb", bufs=1) as sb,
            tc.tile_pool(name="ps", bufs=1, space="PSUM") as ps,
        ):
            blobt = sb.tile([P, BLOBW], _U8, tag="blobt")
            stile = sb.tile([P, NBLK, F], _F8, tag="stile")
            # writeback staging: partition p carries [gc_p | gg_p | net_p] as
            # 72 contiguous f32 (partitions 32..127 are zeroed junk the host
            # ignores); kv_writeback streams the whole [128, 72] block out.
            src4 = sb.tile([P, 1, 1, OW], _F32, tag="src")
            src = src4[:, 0, 0]
            ctxi = sb.tile([P, 1], mybir.dt.int32, tag="ctxi")

            # --- all small inputs in ONE SP HWDGE DMA, hoisted pre-barrier
            # by the post-compile surgery so its transfer (848ns) runs in the
            # SWDGE warm-up window and the stream follows seamlessly ---
            nc.sync.dma_start(out=blobt, in_=blob)

            rowLv = blobt[:, 0:BLOB_ROWL].bitcast(_F8).rearrange(
                "p (b g) -> p b g", b=NBLK)
            smtv = blobt[:, BLOB_ROWL:BLOB_ROWL + BLOB_SMT].bitcast(
                _F16).rearrange("p (nb f) -> p nb f", nb=2)
            colNv = blobt[:, BLOB_ROWL + BLOB_SMT:BLOBW].bitcast(
                _F8).rearrange("p (j k g) -> p j k g", j=QP, k=2)

            # three banks, all at partition base 0
            gcp = ps.tile([G, T], _F32, tag="gcp")
            ggp = ps.tile([G, T], _F32, tag="ggp")
            netp = ps.tile([G, T], _F32, tag="netp")
            nc.vector.memset(src[:], 0.0)
            nc.vector.memset(ctxi[:], 0)

            # --- S stream: fp8 cast DMAs on the SWDGE ring ---
            # partition p = na*C + msub; block blk: n = blk*A + na;
            # free = (q, t) with m = msub*Q + q.
            s_r = s.rearrange("(blk a) (c q) t -> (a c) blk (q t)", a=A, c=C)
            for b0, b1 in DMA_BLOCKS:
                nc.gpsimd.dma_start(
                    out=stile[:, b0:b1, :].rearrange("p b f -> p (b f)"),
                    in_=s_r[:, b0:b1, :])

            # --- output path: descriptors prepared during the stream, the
            # trigger fires them once `src` is fully written ---
            dma_sem = nc.alloc_semaphore("outdma")
            nc.gpsimd.kv_writeback(
                out, src4[:], ctxi[:], prepare_only=True, sem=dma_sem)

            # --- PE pass ---
            # gc|gg projections first in PE program order: smt arrives with
            # the blob (~2us), well before the first S pair is consumable.
            for nb in range(2):
                nc.tensor.matmul(gcp, smtv[:, nb, 0:G],
                                 smtv[:, nb, G:G + T],
                                 start=(nb == 0), stop=(nb == 1),
                                 skip_group_check=True)
                nc.tensor.matmul(ggp, smtv[:, nb, 0:G],
                                 smtv[:, nb, G + T:],
                                 start=(nb == 0), stop=(nb == 1),
                                 skip_group_check=True)

            # One [G, T] accumulation group over all 1024 DoubleRow matmuls:
            # row term adds M[g,n]-projected q-slices (2 blocks per pass),
            # col term adds -M[g,m]-weighted q-pairs (2 q per pass).
            netw = netp
            for pr in range(PAIRS):
                lhs_row = rowLv[:, 2 * pr:2 * pr + 2, :]
                for q in range(Q):
                    nc.tensor.matmul(
                        netw, lhs_row,
                        stile[:, 2 * pr:2 * pr + 2, q * T:(q + 1) * T],
                        start=(pr == 0 and q == 0), stop=False,
                        perf_mode=mybir.MatmulPerfMode.DoubleRow,
                        skip_group_check=True)
                for blk in (2 * pr, 2 * pr + 1):
                    for j in range(QP):
                        nc.tensor.matmul(
                            netw, colNv[:, j],
                            stile[:, blk, 2 * j * T:(2 * j + 2) * T]
                            .rearrange("p (k t) -> p k t", k=2),
                            start=False,
                            stop=(pr == PAIRS - 1 and blk == 2 * pr + 1
                                  and j == QP - 1),
                            perf_mode=mybir.MatmulPerfMode.DoubleRow,
                            skip_group_check=True)

            # --- stage [gc | gg | net] and fire the scatter ---
            # Partition-preserving ACT copies; only the net copy is on the
            # post-stream critical path.
            act_done = nc.alloc_semaphore("actdone")
            nc.vector.tensor_copy(out=src[0:G, 0:T], in_=gcp)
            nc.vector.tensor_copy(out=src[0:G, T:2 * T], in_=ggp)
            nc.vector.tensor_copy(out=src[0:G, 2 * T:3 * T], in_=netp)
            # Placeholder gate (>=0 so the schedule-time sim sails through);
            # post-compile surgery points it at the tile Activation engine sem
            # (ACT instructions cannot carry a second sync update, and the
            # trigger cannot carry a second wait).
            nc.gpsimd.wait_ge(act_done, 0)
            nc.gpsimd.trigger_dma(count=None)
    nc.compile()
    # Drop the framework's const-tensor memsets: nothing reads them, but they
    # run on the Pool engine ahead of the barrier and delay the first SWDGE
    # descriptor emission of the S stream.
    for blk in nc.m.functions[0].blocks:
        blk.instructions = [
            i for i in blk.instructions
            if not (type(i).__name__ == "InstMemset"
                    and i.outs and "const-" in str(i.outs[0]))
        ]
    # Tile schedules the scatter prep on a DMASW lane and the exit drain
    # waits on that lane's sem, but the descriptor-baked completion sem
    # (on_update[0], hardware increments by 16) is the user sem= kwarg.
    # Point on_update[0] at the orphaned DMASW sem so the DMA engines bump
    # the sem the drain actually waits on.
    fn = nc.m.functions[0]
    updated, waited, prep = set(), {}, None
    for blk in fn.blocks:
        for ins in blk.instructions:
            if type(ins).__name__ == "InstKVWritebackAnt":
                prep = ins
            si = ins.sync_info
            if si is None:
                continue
            for u in si.on_update:
                updated.add(u.id)
            for w in si.on_wait:
                waited[w.id] = w
    orphans = [w for wid, w in waited.items()
               if wid not in updated and (w.ant_name or "").startswith("DMASW")]
    assert prep is not None and len(orphans) == 1, (prep, orphans)
    u0 = prep.sync_info.on_update[0]
    assert u0.ant_name == "outdma", u0
    prep.sync_info.on_update[0] = mybir.SyncUpdate(
        sync_type=u0.sync_type, id=orphans[0].id, ant_name=orphans[0].ant_name,
        update_mode=u0.update_mode, update_value=u0.update_value,
        update_reg=u0.update_reg)
    # The sem-assignment pass drops the trigger's cross-engine RAW waits (it
    # only gates on the prep's Pool tick), so the trigger could fire before
    # the staging copies.  The placeholder wait_ge(actdone) sits right before
    # the trigger on the Pool SEQ; point it at the Activation engine-proc sem
    # with the cumulative tick of the last staging copy.
    # The scheduler can linearize the ACT exit drain (which waits on the
    # writeback's DMASW sem) BEFORE the staging copies on the same engine --
    # circular in strict block order.  Move the copies ahead of any
    # instruction waiting on the orphan sem.
    orphan_id = orphans[0].id
    for blk in fn.blocks:
        insts = blk.instructions
        drain_pos = None
        for i, ins in enumerate(insts):
            si = ins.sync_info
            if si and any(w.id == orphan_id for w in si.on_wait):
                drain_pos = i
                break
        if drain_pos is None:
            continue
        late = [ins for ins in insts[drain_pos:]
                if type(ins).__name__ in ("InstActivation", "InstTensorCopy")]
        if late:
            rest = [ins for ins in insts if ins not in late]
            blk.instructions = (rest[:drain_pos] + late + rest[drain_pos:])
    # The framework's ACT table load lands in the postamble AFTER the exit
    # wait on the writeback sem, adding ~1.3us of pure tail.  Hoist it to the
    # head of the main block so it overlaps the stream (baseline behavior).
    loads = []
    for blk in fn.blocks:
        keep = []
        for ins in blk.instructions:
            if type(ins).__name__ == "InstLoadActFuncSet":
                loads.append(ins)
            else:
                keep.append(ins)
        blk.instructions = keep
    if loads:
        main = fn.blocks[1]
        main.instructions = loads + main.instructions
    # The trigger can carry only one codegen sync wait; point it at the ACT
    # engine sem tick of the last staging copy (the prep's descriptor gen on
    # the Pool engine finishes several microseconds earlier, so dropping the
    # Pool tick wait is safe).  Delete the placeholder gate entirely.
    act_total = 0
    last_src_tick = None
    trig = None
    gate = None
    for blk in fn.blocks:
        for ins in blk.instructions:
            if type(ins).__name__ == "InstTriggerDma":
                trig = ins
            si = ins.sync_info
            if si is None:
                continue
            for w in si.on_wait:
                if w.ant_name == "actdone":
                    gate = ins
            for u in si.on_update:
                if (u.ant_name or "").startswith("DVE_"):
                    act_total += (u.update_value or 1)
                    if type(ins).__name__ == "InstTensorCopy":
                        last_src_tick = (u.id, u.ant_name, act_total)
    assert trig is not None and last_src_tick is not None, (trig, last_src_tick)
    sid, sname, val = last_src_tick
    trig.sync_info.on_wait = [mybir.SyncWait(
        sync_type="semaphore", id=sid, ant_name=sname,
        wait_mode="sem-ge-imm", wait_value=val, wait_reg=None)]
    if gate is not None:
        for blk in fn.blocks:
            blk.instructions = [i for i in blk.instructions if i is not gate]
    # Hoist the blob HWDGE DMA (SP) and the first stream DMA (Pool) ahead of
    # the entry barrier: their descriptors have no dependencies, so the first
    # transfer starts ~1.3us in instead of ~2.2us.
    main = fn.blocks[1]
    hoist = []
    seen_pool = seen_sp = False
    keep = []
    for ins in main.instructions:
        if (type(ins).__name__ == "InstDMACopy" and not seen_sp
                and ins.engine == mybir.EngineType.SP):
            hoist.append(ins)
            seen_sp = True
        elif (type(ins).__name__ == "InstDMACopy" and not seen_pool
                and ins.engine == mybir.EngineType.Pool):
            hoist.append(ins)
            seen_pool = True
        else:
            keep.append(ins)
    main.instructions = keep
    fn.blocks[0].instructions = hoist + fn.blocks[0].instructions
    # Exit-barrier trim: the writeback completion is already enforced by the
    # per-engine DMASW waits in the exit block; the trailing gather/release
    # barrier rounds only synchronize engine end times.  Drop them so the
    # kernel ends when the last DMASW waiter releases.
    exit_blk = fn.blocks[-1]
    exit_blk.instructions = [
        i for i in exit_blk.instructions
        if not i.name.startswith("barrier_")
    ]
    return nc


_NC_CACHE = None


def _get_program():
    global _NC_CACHE
    if _NC_CACHE is None:
        _NC_CACHE = _build_program()
    return _NC_CACHE


_RUNNER_CACHE = None


def _get_runner():
    """Compiled-once jit(shard_map) executor over 8 cores."""
    global _RUNNER_CACHE
    if _RUNNER_CACHE is None:
        import jax
        from jax.sharding import Mesh, PartitionSpec
        from jax.experimental.shard_map import shard_map
        from concourse import bass2jax, mybir as mb

        nc = _get_program()
        bass2jax.install_neuronx_cc_hook()
        partition_name = (nc.partition_id_tensor.name
                          if nc.partition_id_tensor else None)
        in_names, out_names, out_avals = [], [], []
        for alloc in nc.m.functions[0].allocations:
            if not isinstance(alloc, mb.MemoryLocationSet):
                continue
            name = alloc.memorylocations[0].name
            if alloc.kind == "ExternalInput":
                if name != partition_name:
                    in_names.append(name)
            elif alloc.kind == "ExternalOutput":
                out_names.append(name)
                out_avals.append(jax.core.ShapedArray(
                    tuple(alloc.tensor_shape), mb.dt.np(alloc.dtype)))
        n_params = len(in_names)
        all_names = in_names + out_names
        if partition_name is not None:
            all_names = all_names + [partition_name]

        def _body(*args):
            operands = list(args)
            if partition_name is not None:
                operands.append(bass2jax.partition_id_tensor())
            outs = bass2jax._bass_exec_p.bind(
                *operands,
                out_avals=tuple(out_avals),
                in_names=tuple(all_names),
                out_names=tuple(out_names),
                lowering_input_output_aliases=(),
                sim_require_finite=True,
                sim_require_nnan=True,
                nc=nc,
            )
            return tuple(outs)

        devices = jax.devices()[:8]
        mesh = Mesh(np.asarray(devices), ("core",))
        n_outs = len(out_names)
        sharded = jax.jit(
            shard_map(_body, mesh=mesh,
                      in_specs=(PartitionSpec("core"),) * (n_params + n_outs),
                      out_specs=(PartitionSpec("core"),) * n_outs,
                      check_rep=False),
            donate_argnums=tuple(range(n_params, n_params + n_outs)),
            keep_unused=True,
        )
        _RUNNER_CACHE = (sharded, in_names[:n_params], out_names, out_avals)
    return _RUNNER_CACHE


def _host_side(consumption, generation, sharing_matrix, lv_group_ids,
               valid_lv_mask):
    """Shared input prep: per-core input maps."""
    consumption = np.ascontiguousarray(consumption, dtype=np.float32)
    generation = np.ascontiguousarray(generation, dtype=np.float32)
    sharing_matrix = np.ascontiguousarray(sharing_matrix, dtype=np.float32)
    ids = np.asarray(lv_group_ids)
    valid = np.asarray(valid_lv_mask, dtype=np.float32)

    onehot = (ids[None, :] == np.arange(G)[:, None]).astype(np.float32)
    n_unique = np.float32(np.unique(ids).size)
    M = onehot * valid[None, :]                      # [G, N]
    mt = np.ascontiguousarray(M.T)                   # [N, G]

    # coln[p=(na,msub), j, k, g] = -M[g, msub*Q + 2j + k]  (negated imports)
    cw = (-mt).reshape(C, Q, G)                      # [msub, q, g]
    coln = np.broadcast_to(cw[None], (A, C, Q, G)).reshape(P, Q * G)
    coln8 = np.ascontiguousarray(
        coln.astype(ml_dtypes.float8_e4m3).view(np.uint8))

    in_maps = []
    for c in range(8):
        b, hh = divmod(c, 2)
        sl = slice(hh * NLOC, (hh + 1) * NLOC)
        mt_half = mt[sl]                             # [NLOC, G]
        # rowL[p=(na,msub), blk, g] = M[g, blk*A + na]
        proj = mt_half.reshape(NBLK, A, G).transpose(1, 0, 2)   # [na, blk, g]
        proj = np.broadcast_to(proj[:, None], (A, C, NBLK, G))
        rowl8 = proj.reshape(P, NBLK * G).astype(
            ml_dtypes.float8_e4m3).view(np.uint8)
        # smt[p, nb, f]: f = [mt row | cons row | gen row] for n = nb*P + p
        sm = np.empty((2, P, SMW), np.float16)
        sm[:, :, :G] = mt_half.reshape(2, P, G)
        sm[:, :, G:G + T] = consumption[b, sl].reshape(2, P, T)
        sm[:, :, G + T:] = generation[b, sl].reshape(2, P, T)
        sm_bytes = sm.transpose(1, 0, 2).reshape(P, -1).view(np.uint8)

        blob = np.zeros((P, BLOBW), np.uint8)
        blob[:, :BLOB_ROWL] = rowl8
        blob[:, BLOB_ROWL:BLOB_ROWL + BLOB_SMT] = sm_bytes
        blob[:, BLOB_ROWL + BLOB_SMT:] = coln8
        in_maps.append({
            "s": np.ascontiguousarray(sharing_matrix[b, sl]),
            "blob": np.ascontiguousarray(blob),
        })
    return in_maps, n_unique


def kernel(consumption, generation, sharing_matrix, lv_group_ids,
           valid_lv_mask, imbalance_penalty_weight, _want_results=False,
           **run_kwargs):
    w = np.float32(np.asarray(imbalance_penalty_weight))
    in_maps, n_unique = _host_side(consumption, generation, sharing_matrix,
                                   lv_group_ids, valid_lv_mask)
    res = None
    if _want_results or run_kwargs:
        nc = _get_program()
        res = run_bass_kernel_spmd(nc, in_maps, core_ids=list(range(8)),
                                   **run_kwargs)
        parts = np.stack([res.results[c]["out"] for c in range(8)])
    else:
        try:
            fn, in_names, out_names, out_avals = _get_runner()
            concat_in = [np.concatenate([m[name] for m in in_maps], axis=0)
                         for name in in_names]
            zeros = [np.zeros((8 * a.shape[0], *a.shape[1:]), a.dtype)
                     for a in out_avals]
            out_arrs = fn(*concat_in, *zeros)
            parts = np.asarray(out_arrs[out_names.index("out")]).reshape(
                8, P, OW)
        except Exception:
            nc = _get_program()
            res = run_bass_kernel_spmd(nc, in_maps, core_ids=list(range(8)))
            parts = np.stack([res.results[c]["out"] for c in range(8)]).reshape(
                8, P, OW)
    # partition p (< G) carries [gc_p | gg_p | net_p] as 3*T columns
    per_core = parts[:, :G, :].reshape(8, G, 3, T).transpose(0, 2, 1, 3)
    full = per_core.reshape(B, 2, 3, G, T).sum(axis=1, dtype=np.float32)
    gc, gg, net = full[:, 0], full[:, 1], full[:, 2]

    imbalance = np.abs(gc - gg + net)
    total = gc + gg + np.float32(EPS)
    pen = np.maximum(imbalance / total - np.float32(TOL), np.float32(0))
    outv = np.float32(pen.sum(dtype=np.float32) * w / n_unique)
    out_arr = np.array(outv, dtype=np.float32)
    if _want_results:
        return out_arr, res
    return out_arr


# revision 64
# speedup vs baseline: 1.2236x; 1.0032x over previous
"""Trainium2 Bass kernel for nn_EnergyBalanceChecker (segment_reduce), v5.

Problem (hardcoded): B=4, N=512, T=24, G=32, TOL=0.05, EPS=1e-6.

  M = onehot(lv_group_ids) * valid_lv_mask                     # [G, N]
  gc  = einsum('gn,bnt->bgt', M, consumption)
  gg  = einsum('gn,bnt->bgt', M, generation)
  net = einsum('gn,bnt->bgt', M, S.sum(axis=2) - S.sum(axis=1))
  pen = relu(|gc-gg+net| / (gc+gg+eps) - TOL);  out = pen.sum()*w/n_unique

Sharding: 8 cores = 4 batches x 2 halves of the (row) N axis.

v5 dataflow (vs v4): the q-axis fold moves INTO the matmul pass, so there
is no wide-PSUM drain at all, and the output leaves via a pre-prepared
SWDGE scatter fired by a trigger instruction:
  * S streams in fp8e4 as before (SWDGE cast DMAs; cost is charged on
    destination bytes).  Partitions carry (na in 8 n-rows) x (msub in 16
    m-blocks); free = (q, t).
  * Row term: per (block-pair, q) a DoubleRow matmul with the M[g,n]
    projection lhsT and a T-wide rhs slice accumulates straight into a
    single [G, T] PSUM tile (q and pairs both fold in PSUM).
  * Col term: per (block, q-pair) a DoubleRow matmul pairs two q slices
    of one block with per-q lhsT columns -M[g, m(msub,q)] -- accumulating
    *negated* imports into the SAME [G, T] tile, so net = row - col needs
    no subtract, just one PSUM->SBUF copy at the end.
  * gc|gg from a small f16 side input (one matmul pair, mid-stream).
  * Output: a prepare_only dma_scatter_add writes descriptors during the
    stream; a Pool trigger_dma fires them once the [G,4,T] staging tile is
    ready -- skipping the whole HWDGE SEQ/gen/delay chain at the tail.
  * Host does only the [3, G, T]-level nonlinear tail.
"""

import sys

import numpy as np

try:
    import concourse  # noqa: F401
except ImportError:
    sys.path.insert(0, "/opt/trn_rl_repo")

import ml_dtypes

import concourse.tile as tile
from concourse import bacc, mybir
from concourse.bass_utils import run_bass_kernel_spmd

B, N, T, G = 4, 512, 24, 32
TOL, EPS = 0.05, 1e-6
P = 128                 # SBUF partitions
NLOC = N // 2           # rows per core (n-half)
A = 8                   # n-rows per block (partition sub-dim)
C = 16                  # m-blocks on partitions (partition sub-dim)
Q = N // C              # m-columns per msub block (free dim)
QP = Q // 2             # q-pairs for the col-term matmuls
NBLK = NLOC // A        # 32 blocks of 8 n-rows
PAIRS = NBLK // 2       # DoubleRow pairs
F = Q * T               # free elements per block
DMA_BLOCKS = ((0, 5), (5, 11), (11, 17), (17, 23), (23, 28),
              (28, 30), (30, 32))   # stream DMA block ranges
SMW = G + 2 * T         # smt row: [mt | cons | gen] per n
BLOB_ROWL = NBLK * G                   # 1024 u8 per partition
BLOB_SMT = 2 * SMW * 2                 # 320 u8 (f16) per partition
BLOBW = BLOB_ROWL + BLOB_SMT
CNW = QP * 2 * G                       # 1024 u8 colN payload (16 partitions)
CNSW = CNW + P                         # + 128 u8 replicate indicator
OW = 3 * T              # out row: [gc | gg | net] per group

_F32 = mybir.dt.float32
_F16 = mybir.dt.float16
_F8 = mybir.dt.float8e4
_U8 = mybir.dt.uint8
_I16 = mybir.dt.int16


def _build_program():
    nc = bacc.Bacc("TRN2", target_bir_lowering=False, debug=False,
                   enable_asserts=False, num_devices=8)
    s = nc.dram_tensor("s", [NLOC, N, T], _F32, kind="ExternalInput").ap()
    blob = nc.dram_tensor("blob", [P, BLOBW], _U8, kind="ExternalInput").ap()
    cns = nc.dram_tensor("cns", [C, CNSW], _U8, kind="ExternalInput").ap()
    # kv_writeback layout: [batch, d_head_inner, d_head_outer, n_ctx]
    out = nc.dram_tensor("out", [1, P, 1, OW], _F32, kind="ExternalOutput").ap()

    with tile.TileContext(nc) as tc, nc.allow_low_precision(
            "fp8 S stream + fp8 {0,1} masks, f32 PSUM accumulation"):
        with (
            tc.tile_pool(name="sb", bufs=1) as sb,
            tc.tile_pool(name="ps", bufs=1, space="PSUM") as ps,
        ):
            blobt = sb.tile([P, BLOBW], _U8, tag="blobt")
            stile = sb.tile([P, NBLK, F], _F8, tag="stile")
            # writeback staging: partition p carries [gc_p | gg_p | net_p] as
            # 72 contiguous f32 (partitions 32..127 are zeroed junk the host
            # ignores); kv_writeback streams the whole [128, 72] block out.
            src4 = sb.tile([P, 1, 1, OW], _F32, tag="src")
            src = src4[:, 0, 0]
            ctxi = sb.tile([P, 1], mybir.dt.int32, tag="ctxi")

            # --- small inputs on two SP HWDGE DMAs, hoisted pre-barrier by
            # the post-compile surgery so their transfers (~530ns) run in the
            # SWDGE warm-up window and the stream follows seamlessly.  colN
            # ships once (16 partitions) and is replicated to 128 partitions
            # on-device via an indicator matmul + ACT cast-copies. ---
            cnst = sb.tile([C, CNSW], _U8, tag="cnst")
            colnt = sb.tile([P, CNW], _F8, tag="colnt")
            nc.sync.dma_start(out=blobt, in_=blob)
            nc.sync.dma_start(out=cnst, in_=cns)

            rowLv = blobt[:, 0:BLOB_ROWL].bitcast(_F8).rearrange(
                "p (b g) -> p b g", b=NBLK)
            smtv = blobt[:, BLOB_ROWL:BLOB_ROWL + BLOB_SMT].bitcast(
                _F16).rearrange("p (nb f) -> p nb f", nb=2)
            cnv = cnst[:, 0:CNW].bitcast(_F8)
            indv = cnst[:, CNW:CNSW].bitcast(_F8)
            colNv = colnt.rearrange("p (j k g) -> p j k g", j=QP, k=2)

            # three banks, all at partition base 0
            gcp = ps.tile([G, T], _F32, tag="gcp")
            ggp = ps.tile([G, T], _F32, tag="ggp")
            netp = ps.tile([G, T], _F32, tag="netp")
            repa = ps.tile([P, CNW // 2], _F32, tag="repa")
            repb = ps.tile([P, CNW // 2], _F32, tag="repb")
            nc.vector.memset(src[:], 0.0)
            nc.vector.memset(ctxi[:], 0)

            # --- S stream: fp8 cast DMAs on the SWDGE ring ---
            # partition p = na*C + msub; block blk: n = blk*A + na;
            # free = (q, t) with m = msub*Q + q.
            s_r = s.rearrange("(blk a) (c q) t -> (a c) blk (q t)", a=A, c=C)
            for b0, b1 in DMA_BLOCKS:
                nc.gpsimd.dma_start(
                    out=stile[:, b0:b1, :].rearrange("p b f -> p (b f)"),
                    in_=s_r[:, b0:b1, :])

            # --- output path: descriptors prepared during the stream, the
            # trigger fires them once `src` is fully written ---
            dma_sem = nc.alloc_semaphore("outdma")
            nc.gpsimd.kv_writeback(
                out, src4[:], ctxi[:], prepare_only=True, sem=dma_sem)

            # --- PE pass ---
            # colN replication first: out[pp, f] = cn[pp % 16, f] via the
            # [16, 128] indicator lhsT; ACT cast-copies land it as fp8.
            H = CNW // 2
            nc.tensor.matmul(repa, indv, cnv[:, 0:H],
                             start=True, stop=True, skip_group_check=True)
            nc.tensor.matmul(repb, indv, cnv[:, H:],
                             start=True, stop=True, skip_group_check=True)
            nc.scalar.copy(out=colnt[:, 0:H], in_=repa)
            nc.scalar.copy(out=colnt[:, H:], in_=repb)
            # gc|gg projections next in PE program order: smt arrives with
            # the blob (~2us), well before the first S pair is consumable.
            for nb in range(2):
                nc.tensor.matmul(gcp, smtv[:, nb, 0:G],
                                 smtv[:, nb, G:G + T],
                                 start=(nb == 0), stop=(nb == 1),
                                 skip_group_check=True)
                nc.tensor.matmul(ggp, smtv[:, nb, 0:G],
                                 smtv[:, nb, G + T:],
                                 start=(nb == 0), stop=(nb == 1),
                                 skip_group_check=True)

            # One [G, T] accumulation group over all 1024 DoubleRow matmuls:
            # row term adds M[g,n]-projected q-slices (2 blocks per pass),
            # col term adds -M[g,m]-weighted q-pairs (2 q per pass).
            netw = netp
            for pr in range(PAIRS):
                lhs_row = rowLv[:, 2 * pr:2 * pr + 2, :]
                for q in range(Q):
                    nc.tensor.matmul(
                        netw, lhs_row,
                        stile[:, 2 * pr:2 * pr + 2, q * T:(q + 1) * T],
                        start=(pr == 0 and q == 0), stop=False,
                        perf_mode=mybir.MatmulPerfMode.DoubleRow,
                        skip_group_check=True)
                for blk in (2 * pr, 2 * pr + 1):
                    for j in range(QP):
                        nc.tensor.matmul(
                            netw, colNv[:, j],
                            stile[:, blk, 2 * j * T:(2 * j + 2) * T]
                            .rearrange("p (k t) -> p k t", k=2),
                            start=False,
                            stop=(pr == PAIRS - 1 and blk == 2 * pr + 1
                                  and j == QP - 1),
                            perf_mode=mybir.MatmulPerfMode.DoubleRow,
                            skip_group_check=True)

            # --- stage [gc | gg | net] and fire the scatter ---
            # Partition-preserving ACT copies; only the net copy is on the
            # post-stream critical path.
            act_done = nc.alloc_semaphore("actdone")
            nc.vector.tensor_copy(out=src[0:G, 0:T], in_=gcp)
            nc.vector.tensor_copy(out=src[0:G, T:2 * T], in_=ggp)
            nc.vector.tensor_copy(out=src[0:G, 2 * T:3 * T], in_=netp)
            # Placeholder gate (>=0 so the schedule-time sim sails through);
            # post-compile surgery points it at the tile Activation engine sem
            # (ACT instructions cannot carry a second sync update, and the
            # trigger cannot carry a second wait).
            nc.gpsimd.wait_ge(act_done, 0)
            nc.gpsimd.trigger_dma(count=None)
    nc.compile()
    # Drop the framework's const-tensor memsets: nothing reads them, but they
    # run on the Pool engine ahead of the barrier and delay the first SWDGE
    # descriptor emission of the S stream.
    for blk in nc.m.functions[0].blocks:
        blk.instructions = [
            i for i in blk.instructions
            if not (type(i).__name__ == "InstMemset"
                    and i.outs and "const-" in str(i.outs[0]))
        ]
    # Tile schedules the scatter prep on a DMASW lane and the exit drain
    # waits on that lane's sem, but the descriptor-baked completion sem
    # (on_update[0], hardware increments by 16) is the user sem= kwarg.
    # Point on_update[0] at the orphaned DMASW sem so the DMA engines bump
    # the sem the drain actually waits on.
    fn = nc.m.functions[0]
    updated, waited, prep = set(), {}, None
    for blk in fn.blocks:
        for ins in blk.instructions:
            if type(ins).__name__ == "InstKVWritebackAnt":
                prep = ins
            si = ins.sync_info
            if si is None:
                continue
            for u in si.on_update:
                updated.add(u.id)
            for w in si.on_wait:
                waited[w.id] = w
    orphans = [w for wid, w in waited.items()
               if wid not in updated and (w.ant_name or "").startswith("DMASW")]
    assert prep is not None and len(orphans) == 1, (prep, orphans)
    u0 = prep.sync_info.on_update[0]
    assert u0.ant_name == "outdma", u0
    prep.sync_info.on_update[0] = mybir.SyncUpdate(
        sync_type=u0.sync_type, id=orphans[0].id, ant_name=orphans[0].ant_name,
        update_mode=u0.update_mode, update_value=u0.update_value,
        update_reg=u0.update_reg)
    # The sem-assignment pass drops the trigger's cross-engine RAW waits (it
    # only gates on the prep's Pool tick), so the trigger could fire before
    # the staging copies.  The placeholder wait_ge(actdone) sits right before
    # the trigger on the Pool SEQ; point it at the Activation engine-proc sem
    # with the cumulative tick of the last staging copy.
    # The scheduler can linearize the ACT exit drain (which waits on the
    # writeback's DMASW sem) BEFORE the staging copies on the same engine --
    # circular in strict block order.  Move the copies ahead of any
    # instruction waiting on the orphan sem.
    orphan_id = orphans[0].id
    for blk in fn.blocks:
        insts = blk.instructions
        drain_pos = None
        for i, ins in enumerate(insts):
            si = ins.sync_info
            if si and any(w.id == orphan_id for w in si.on_wait):
                drain_pos = i
                break
        if drain_pos is None:
            continue
        late = [ins for ins in insts[drain_pos:]
                if type(ins).__name__ in ("InstActivation", "InstTensorCopy")]
        if late:
            rest = [ins for ins in insts if ins not in late]
            blk.instructions = (rest[:drain_pos] + late + rest[drain_pos:])
    # The framework's ACT table load lands in the postamble AFTER the exit
    # wait on the writeback sem, adding ~1.3us of pure tail.  Hoist it to the
    # head of the main block so it overlaps the stream (baseline behavior).
    loads = []
    for blk in fn.blocks:
        keep = []
        for ins in blk.instructions:
            if type(ins).__name__ == "InstLoadActFuncSet":
                loads.append(ins)
            else:
                keep.append(ins)
        blk.instructions = keep
    if loads:
        main = fn.blocks[1]
        main.instructions = loads + main.instructions
    # The trigger can carry only one codegen sync wait; point it at the ACT
    # engine sem tick of the last staging copy (the prep's descriptor gen on
    # the Pool engine finishes several microseconds earlier, so dropping the
    # Pool tick wait is safe).  Delete the placeholder gate entirely.
    act_total = 0
    last_src_tick = None
    trig = None
    gate = None
    for blk in fn.blocks:
        for ins in blk.instructions:
            if type(ins).__name__ == "InstTriggerDma":
                trig = ins
            si = ins.sync_info
            if si is None:
                continue
            for w in si.on_wait:
                if w.ant_name == "actdone":
                    gate = ins
            for u in si.on_update:
                if (u.ant_name or "").startswith("DVE_"):
                    act_total += (u.update_value or 1)
                    if type(ins).__name__ == "InstTensorCopy":
                        last_src_tick = (u.id, u.ant_name, act_total)
    assert trig is not None and last_src_tick is not None, (trig, last_src_tick)
    sid, sname, val = last_src_tick
    trig.sync_info.on_wait = [mybir.SyncWait(
        sync_type="semaphore", id=sid, ant_name=sname,
        wait_mode="sem-ge-imm", wait_value=val, wait_reg=None)]
    if gate is not None:
        for blk in fn.blocks:
            blk.instructions = [i for i in blk.instructions if i is not gate]
    # Hoist the blob HWDGE DMA (SP) and the first stream DMA (Pool) ahead of
    # the entry barrier: their descriptors have no dependencies, so the first
    # transfer starts ~1.3us in instead of ~2.2us.
    main = fn.blocks[1]
    hoist = []
    n_sp = n_pool = 0
    keep = []
    for ins in main.instructions:
        if (type(ins).__name__ == "InstDMACopy" and n_sp < 2
                and ins.engine == mybir.EngineType.SP):
            hoist.append(ins)
            n_sp += 1
        elif (type(ins).__name__ == "InstDMACopy" and n_pool < 1
                and ins.engine == mybir.EngineType.Pool):
            hoist.append(ins)
            n_pool += 1
        else:
            keep.append(ins)
    main.instructions = keep
    fn.blocks[0].instructions = hoist + fn.blocks[0].instructions
    # Exit-barrier trim: the writeback completion is already enforced by the
    # per-engine DMASW waits in the exit block; the trailing gather/release
    # barrier rounds only synchronize engine end times.  Drop them so the
    # kernel ends when the last DMASW waiter releases.
    exit_blk = fn.blocks[-1]
    exit_blk.instructions = [
        i for i in exit_blk.instructions
        if not i.name.startswith("barrier_")
    ]
    return nc


_NC_CACHE = None


def _get_program():
    global _NC_CACHE
    if _NC_CACHE is None:
        _NC_CACHE = _build_program()
    return _NC_CACHE


_RUNNER_CACHE = None


def _get_runner():
    """Compiled-once jit(shard_map) executor over 8 cores."""
    global _RUNNER_CACHE
    if _RUNNER_CACHE is None:
        import jax
        from jax.sharding import Mesh, PartitionSpec
        from jax.experimental.shard_map import shard_map
        from concourse import bass2jax, mybir as mb

        nc = _get_program()
        bass2jax.install_neuronx_cc_hook()
        partition_name = (nc.partition_id_tensor.name
                          if nc.partition_id_tensor else None)
        in_names, out_names, out_avals = [], [], []
        for alloc in nc.m.functions[0].allocations:
            if not isinstance(alloc, mb.MemoryLocationSet):
                continue
            name = alloc.memorylocations[0].name
            if alloc.kind == "ExternalInput":
                if name != partition_name:
                    in_names.append(name)
            elif alloc.kind == "ExternalOutput":
                out_names.append(name)
                out_avals.append(jax.core.ShapedArray(
                    tuple(alloc.tensor_shape), mb.dt.np(alloc.dtype)))
        n_params = len(in_names)
        all_names = in_names + out_names
        if partition_name is not None:
            all_names = all_names + [partition_name]

        def _body(*args):
            operands = list(args)
            if partition_name is not None:
                operands.append(bass2jax.partition_id_tensor())
            outs = bass2jax._bass_exec_p.bind(
                *operands,
                out_avals=tuple(out_avals),
                in_names=tuple(all_names),
                out_names=tuple(out_names),
                lowering_input_output_aliases=(),
                sim_require_finite=True,
                sim_require_nnan=True,
                nc=nc,
            )
            return tuple(outs)

        devices = jax.devices()[:8]
        mesh = Mesh(np.asarray(devices), ("core",))
        n_outs = len(out_names)
        sharded = jax.jit(
            shard_map(_body, mesh=mesh,
                      in_specs=(PartitionSpec("core"),) * (n_params + n_outs),
                      out_specs=(PartitionSpec("core"),) * n_outs,
                      check_rep=False),
            donate_argnums=tuple(range(n_params, n_params + n_outs)),
            keep_unused=True,
        )
        _RUNNER_CACHE = (sharded, in_names[:n_params], out_names, out_avals)
    return _RUNNER_CACHE


def _host_side(consumption, generation, sharing_matrix, lv_group_ids,
               valid_lv_mask):
    """Shared input prep: per-core input maps."""
    consumption = np.ascontiguousarray(consumption, dtype=np.float32)
    generation = np.ascontiguousarray(generation, dtype=np.float32)
    sharing_matrix = np.ascontiguousarray(sharing_matrix, dtype=np.float32)
    ids = np.asarray(lv_group_ids)
    valid = np.asarray(valid_lv_mask, dtype=np.float32)

    onehot = (ids[None, :] == np.arange(G)[:, None]).astype(np.float32)
    n_unique = np.float32(np.unique(ids).size)
    M = onehot * valid[None, :]                      # [G, N]
    mt = np.ascontiguousarray(M.T)                   # [N, G]

    # cns[msub, j, k, g] = -M[g, msub*Q + 2j + k]  (negated imports), plus
    # the [16, 128] replicate indicator ind[p, pp] = (pp % 16 == p).
    cw = (-mt).reshape(C, Q * G)                     # [msub, (q, g)]
    cn16 = cw.astype(ml_dtypes.float8_e4m3).view(np.uint8)
    ind = (np.arange(P)[None, :] % C == np.arange(C)[:, None]).astype(
        np.float32).astype(ml_dtypes.float8_e4m3).view(np.uint8)
    cns = np.ascontiguousarray(
        np.concatenate([cn16, ind], axis=1))         # [16, CNSW]

    in_maps = []
    for c in range(8):
        b, hh = divmod(c, 2)
        sl = slice(hh * NLOC, (hh + 1) * NLOC)
        mt_half = mt[sl]                             # [NLOC, G]
        # rowL[p=(na,msub), blk, g] = M[g, blk*A + na]
        proj = mt_half.reshape(NBLK, A, G).transpose(1, 0, 2)   # [na, blk, g]
        proj = np.broadcast_to(proj[:, None], (A, C, NBLK, G))
        rowl8 = proj.reshape(P, NBLK * G).astype(
            ml_dtypes.float8_e4m3).view(np.uint8)
        # smt[p, nb, f]: f = [mt row | cons row | gen row] for n = nb*P + p
        sm = np.empty((2, P, SMW), np.float16)
        sm[:, :, :G] = mt_half.reshape(2, P, G)
        sm[:, :, G:G + T] = consumption[b, sl].reshape(2, P, T)
        sm[:, :, G + T:] = generation[b, sl].reshape(2, P, T)
        sm_bytes = sm.transpose(1, 0, 2).reshape(P, -1).view(np.uint8)

        blob = np.zeros((P, BLOBW), np.uint8)
        blob[:, :BLOB_ROWL] = rowl8
        blob[:, BLOB_ROWL:BLOB_ROWL + BLOB_SMT] = sm_bytes
        in_maps.append({
            "s": np.ascontiguousarray(sharing_matrix[b, sl]),
            "blob": np.ascontiguousarray(blob),
            "cns": cns,
        })
    return in_maps, n_unique


def kernel(consumption, generation, sharing_matrix, lv_group_ids,
           valid_lv_mask, imbalance_penalty_weight, _want_results=False,
           **run_kwargs):
    w = np.float32(np.asarray(imbalance_penalty_weight))
    in_maps, n_unique = _host_side(consumption, generation, sharing_matrix,
                                   lv_group_ids, valid_lv_mask)
    res = None
    if _want_results or run_kwargs:
        nc = _get_program()
        res = run_bass_kernel_spmd(nc, in_maps, core_ids=list(range(8)),
                                   **run_kwargs)
        parts = np.stack([res.results[c]["out"] for c in range(8)])
    else:
        try:
            fn, in_names, out_names, out_avals = _get_runner()
            concat_in = [np.concatenate([m[name] for m in in_maps], axis=0)
                         for name in in_names]
            zeros = [np.zeros((8 * a.shape[0], *a.shape[1:]), a.dtype)
                     for a in out_avals]
            out_arrs = fn(*concat_in, *zeros)
            parts = np.asarray(out_arrs[out_names.index("out")]).reshape(
                8, P, OW)
        except Exception:
            nc = _get_program()
            res = run_bass_kernel_spmd(nc, in_maps, core_ids=list(range(8)))
            parts = np.stack([res.results[c]["out"] for c in range(8)]).reshape(
                8, P, OW)
    # partition p (< G) carries [gc_p | gg_p | net_p] as 3*T columns
    per_core = parts[:, :G, :].reshape(8, G, 3, T).transpose(0, 2, 1, 3)
    full = per_core.reshape(B, 2, 3, G, T).sum(axis=1, dtype=np.float32)
    gc, gg, net = full[:, 0], full[:, 1], full[:, 2]

    imbalance = np.abs(gc - gg + net)
    total = gc + gg + np.float32(EPS)
    pen = np.maximum(imbalance / total - np.float32(TOL), np.float32(0))
    outv = np.float32(pen.sum(dtype=np.float32) * w / n_unique)
    out_arr = np.array(outv, dtype=np.float32)
    if _want_results:
        return out_arr, res
    return out_arr


# revision 65
# speedup vs baseline: 1.2443x; 1.0169x over previous
"""Trainium2 Bass kernel for nn_EnergyBalanceChecker (segment_reduce), v5.

Problem (hardcoded): B=4, N=512, T=24, G=32, TOL=0.05, EPS=1e-6.

  M = onehot(lv_group_ids) * valid_lv_mask                     # [G, N]
  gc  = einsum('gn,bnt->bgt', M, consumption)
  gg  = einsum('gn,bnt->bgt', M, generation)
  net = einsum('gn,bnt->bgt', M, S.sum(axis=2) - S.sum(axis=1))
  pen = relu(|gc-gg+net| / (gc+gg+eps) - TOL);  out = pen.sum()*w/n_unique

Sharding: 8 cores = 4 batches x 2 halves of the (row) N axis.

v5 dataflow (vs v4): the q-axis fold moves INTO the matmul pass, so there
is no wide-PSUM drain at all, and the output leaves via a pre-prepared
SWDGE scatter fired by a trigger instruction:
  * S streams in fp8e4 as before (SWDGE cast DMAs; cost is charged on
    destination bytes).  Partitions carry (na in 8 n-rows) x (msub in 16
    m-blocks); free = (q, t).
  * Row term: per (block-pair, q) a DoubleRow matmul with the M[g,n]
    projection lhsT and a T-wide rhs slice accumulates straight into a
    single [G, T] PSUM tile (q and pairs both fold in PSUM).
  * Col term: per (block, q-pair) a DoubleRow matmul pairs two q slices
    of one block with per-q lhsT columns -M[g, m(msub,q)] -- accumulating
    *negated* imports into the SAME [G, T] tile, so net = row - col needs
    no subtract, just one PSUM->SBUF copy at the end.
  * gc|gg from a small f16 side input (one matmul pair, mid-stream).
  * Output: a prepare_only dma_scatter_add writes descriptors during the
    stream; a Pool trigger_dma fires them once the [G,4,T] staging tile is
    ready -- skipping the whole HWDGE SEQ/gen/delay chain at the tail.
  * Host does only the [3, G, T]-level nonlinear tail.
"""

import sys

import numpy as np

try:
    import concourse  # noqa: F401
except ImportError:
    sys.path.insert(0, "/opt/trn_rl_repo")

import ml_dtypes

import concourse.tile as tile
from concourse import bacc, mybir
from concourse.bass_utils import run_bass_kernel_spmd

B, N, T, G = 4, 512, 24, 32
TOL, EPS = 0.05, 1e-6
P = 128                 # SBUF partitions
NLOC = N // 2           # rows per core (n-half)
A = 8                   # n-rows per block (partition sub-dim)
C = 16                  # m-blocks on partitions (partition sub-dim)
Q = N // C              # m-columns per msub block (free dim)
QP = Q // 2             # q-pairs for the col-term matmuls
NBLK = NLOC // A        # 32 blocks of 8 n-rows
PAIRS = NBLK // 2       # DoubleRow pairs
F = Q * T               # free elements per block
DMA_BLOCKS = ((0, 8), (8, 13), (13, 19), (19, 24), (24, 28),
              (28, 30), (30, 32))   # stream DMA block ranges; the first is
                                    # large so transfers cover gen latency
SMW = G + 2 * T         # smt row: [mt | cons | gen] per n
BLOB_ROWL = NBLK * G                   # 1024 u8 per partition
BLOB_SMT = 2 * SMW * 2                 # 320 u8 (f16) per partition
BLOBW = BLOB_ROWL + BLOB_SMT
CNW = QP * 2 * G                       # 1024 u8 colN payload (16 partitions)
CNSW = CNW + P                         # + 128 u8 replicate indicator
OW = 3 * T              # out row: [gc | gg | net] per group

_F32 = mybir.dt.float32
_F16 = mybir.dt.float16
_F8 = mybir.dt.float8e4
_U8 = mybir.dt.uint8
_I16 = mybir.dt.int16


def _build_program():
    nc = bacc.Bacc("TRN2", target_bir_lowering=False, debug=False,
                   enable_asserts=False, num_devices=8)
    s = nc.dram_tensor("s", [NLOC, N, T], _F32, kind="ExternalInput").ap()
    blob = nc.dram_tensor("blob", [P, BLOBW], _U8, kind="ExternalInput").ap()
    cns = nc.dram_tensor("cns", [C, CNSW], _U8, kind="ExternalInput").ap()
    # kv_writeback layout: [batch, d_head_inner, d_head_outer, n_ctx]
    out = nc.dram_tensor("out", [1, P, 1, OW], _F32, kind="ExternalOutput").ap()

    with tile.TileContext(nc) as tc, nc.allow_low_precision(
            "fp8 S stream + fp8 {0,1} masks, f32 PSUM accumulation"):
        with (
            tc.tile_pool(name="sb", bufs=1) as sb,
            tc.tile_pool(name="ps", bufs=1, space="PSUM") as ps,
        ):
            blobt = sb.tile([P, BLOBW], _U8, tag="blobt")
            stile = sb.tile([P, NBLK, F], _F8, tag="stile")
            # writeback staging: partition p carries [gc_p | gg_p | net_p] as
            # 72 contiguous f32 (partitions 32..127 are zeroed junk the host
            # ignores); kv_writeback streams the whole [128, 72] block out.
            src4 = sb.tile([P, 1, 1, OW], _F32, tag="src")
            src = src4[:, 0, 0]
            ctxi = sb.tile([P, 1], mybir.dt.int32, tag="ctxi")

            # --- small inputs on two SP HWDGE DMAs, hoisted pre-barrier by
            # the post-compile surgery so their transfers (~530ns) run in the
            # SWDGE warm-up window and the stream follows seamlessly.  colN
            # ships once (16 partitions) and is replicated to 128 partitions
            # on-device via an indicator matmul + ACT cast-copies. ---
            cnst = sb.tile([C, CNSW], _U8, tag="cnst")
            colnt = sb.tile([P, CNW], _F8, tag="colnt")
            nc.sync.dma_start(out=blobt, in_=blob)
            nc.sync.dma_start(out=cnst, in_=cns)

            rowLv = blobt[:, 0:BLOB_ROWL].bitcast(_F8).rearrange(
                "p (b g) -> p b g", b=NBLK)
            smtv = blobt[:, BLOB_ROWL:BLOB_ROWL + BLOB_SMT].bitcast(
                _F16).rearrange("p (nb f) -> p nb f", nb=2)
            cnv = cnst[:, 0:CNW].bitcast(_F8)
            indv = cnst[:, CNW:CNSW].bitcast(_F8)
            colNv = colnt.rearrange("p (j k g) -> p j k g", j=QP, k=2)

            # three banks, all at partition base 0
            gcp = ps.tile([G, T], _F32, tag="gcp")
            ggp = ps.tile([G, T], _F32, tag="ggp")
            netp = ps.tile([G, T], _F32, tag="netp")
            repa = ps.tile([P, CNW // 2], _F32, tag="repa")
            repb = ps.tile([P, CNW // 2], _F32, tag="repb")
            nc.vector.memset(src[:], 0.0)
            nc.vector.memset(ctxi[:], 0)

            # --- S stream: fp8 cast DMAs on the SWDGE ring ---
            # partition p = na*C + msub; block blk: n = blk*A + na;
            # free = (q, t) with m = msub*Q + q.
            s_r = s.rearrange("(blk a) (c q) t -> (a c) blk (q t)", a=A, c=C)
            for b0, b1 in DMA_BLOCKS:
                nc.gpsimd.dma_start(
                    out=stile[:, b0:b1, :].rearrange("p b f -> p (b f)"),
                    in_=s_r[:, b0:b1, :])

            # --- output path: descriptors prepared during the stream, the
            # trigger fires them once `src` is fully written ---
            dma_sem = nc.alloc_semaphore("outdma")
            nc.gpsimd.kv_writeback(
                out, src4[:], ctxi[:], prepare_only=True, sem=dma_sem)

            # --- PE pass ---
            # colN replication first: out[pp, f] = cn[pp % 16, f] via the
            # [16, 128] indicator lhsT; ACT cast-copies land it as fp8.
            H = CNW // 2
            nc.tensor.matmul(repa, indv, cnv[:, 0:H],
                             start=True, stop=True, skip_group_check=True)
            nc.tensor.matmul(repb, indv, cnv[:, H:],
                             start=True, stop=True, skip_group_check=True)
            nc.scalar.copy(out=colnt[:, 0:H], in_=repa)
            nc.scalar.copy(out=colnt[:, H:], in_=repb)
            # gc|gg projections next in PE program order: smt arrives with
            # the blob (~2us), well before the first S pair is consumable.
            for nb in range(2):
                nc.tensor.matmul(gcp, smtv[:, nb, 0:G],
                                 smtv[:, nb, G:G + T],
                                 start=(nb == 0), stop=(nb == 1),
                                 skip_group_check=True)
                nc.tensor.matmul(ggp, smtv[:, nb, 0:G],
                                 smtv[:, nb, G + T:],
                                 start=(nb == 0), stop=(nb == 1),
                                 skip_group_check=True)

            # One [G, T] accumulation group over all 1024 DoubleRow matmuls:
            # row term adds M[g,n]-projected q-slices (2 blocks per pass),
            # col term adds -M[g,m]-weighted q-pairs (2 q per pass).
            netw = netp
            for pr in range(PAIRS):
                lhs_row = rowLv[:, 2 * pr:2 * pr + 2, :]
                for q in range(Q):
                    nc.tensor.matmul(
                        netw, lhs_row,
                        stile[:, 2 * pr:2 * pr + 2, q * T:(q + 1) * T],
                        start=(pr == 0 and q == 0), stop=False,
                        perf_mode=mybir.MatmulPerfMode.DoubleRow,
                        skip_group_check=True)
                for blk in (2 * pr, 2 * pr + 1):
                    for j in range(QP):
                        nc.tensor.matmul(
                            netw, colNv[:, j],
                            stile[:, blk, 2 * j * T:(2 * j + 2) * T]
                            .rearrange("p (k t) -> p k t", k=2),
                            start=False,
                            stop=(pr == PAIRS - 1 and blk == 2 * pr + 1
                                  and j == QP - 1),
                            perf_mode=mybir.MatmulPerfMode.DoubleRow,
                            skip_group_check=True)

            # --- stage [gc | gg | net] and fire the scatter ---
            # Partition-preserving ACT copies; only the net copy is on the
            # post-stream critical path.
            act_done = nc.alloc_semaphore("actdone")
            nc.vector.tensor_copy(out=src[0:G, 0:T], in_=gcp)
            nc.vector.tensor_copy(out=src[0:G, T:2 * T], in_=ggp)
            nc.vector.tensor_copy(out=src[0:G, 2 * T:3 * T], in_=netp)
            # Placeholder gate (>=0 so the schedule-time sim sails through);
            # post-compile surgery points it at the tile Activation engine sem
            # (ACT instructions cannot carry a second sync update, and the
            # trigger cannot carry a second wait).
            nc.gpsimd.wait_ge(act_done, 0)
            nc.gpsimd.trigger_dma(count=None)
    nc.compile()
    # Drop the framework's const-tensor memsets: nothing reads them, but they
    # run on the Pool engine ahead of the barrier and delay the first SWDGE
    # descriptor emission of the S stream.
    for blk in nc.m.functions[0].blocks:
        blk.instructions = [
            i for i in blk.instructions
            if not (type(i).__name__ == "InstMemset"
                    and i.outs and "const-" in str(i.outs[0]))
        ]
    # Tile schedules the scatter prep on a DMASW lane and the exit drain
    # waits on that lane's sem, but the descriptor-baked completion sem
    # (on_update[0], hardware increments by 16) is the user sem= kwarg.
    # Point on_update[0] at the orphaned DMASW sem so the DMA engines bump
    # the sem the drain actually waits on.
    fn = nc.m.functions[0]
    updated, waited, prep = set(), {}, None
    for blk in fn.blocks:
        for ins in blk.instructions:
            if type(ins).__name__ == "InstKVWritebackAnt":
                prep = ins
            si = ins.sync_info
            if si is None:
                continue
            for u in si.on_update:
                updated.add(u.id)
            for w in si.on_wait:
                waited[w.id] = w
    orphans = [w for wid, w in waited.items()
               if wid not in updated and (w.ant_name or "").startswith("DMASW")]
    assert prep is not None and len(orphans) == 1, (prep, orphans)
    u0 = prep.sync_info.on_update[0]
    assert u0.ant_name == "outdma", u0
    prep.sync_info.on_update[0] = mybir.SyncUpdate(
        sync_type=u0.sync_type, id=orphans[0].id, ant_name=orphans[0].ant_name,
        update_mode=u0.update_mode, update_value=u0.update_value,
        update_reg=u0.update_reg)
    # The sem-assignment pass drops the trigger's cross-engine RAW waits (it
    # only gates on the prep's Pool tick), so the trigger could fire before
    # the staging copies.  The placeholder wait_ge(actdone) sits right before
    # the trigger on the Pool SEQ; point it at the Activation engine-proc sem
    # with the cumulative tick of the last staging copy.
    # The scheduler can linearize the ACT exit drain (which waits on the
    # writeback's DMASW sem) BEFORE the staging copies on the same engine --
    # circular in strict block order.  Move the copies ahead of any
    # instruction waiting on the orphan sem.
    orphan_id = orphans[0].id
    for blk in fn.blocks:
        insts = blk.instructions
        drain_pos = None
        for i, ins in enumerate(insts):
            si = ins.sync_info
            if si and any(w.id == orphan_id for w in si.on_wait):
                drain_pos = i
                break
        if drain_pos is None:
            continue
        late = [ins for ins in insts[drain_pos:]
                if type(ins).__name__ in ("InstActivation", "InstTensorCopy")]
        if late:
            rest = [ins for ins in insts if ins not in late]
            blk.instructions = (rest[:drain_pos] + late + rest[drain_pos:])
    # The framework's ACT table load lands in the postamble AFTER the exit
    # wait on the writeback sem, adding ~1.3us of pure tail.  Hoist it to the
    # head of the main block so it overlaps the stream (baseline behavior).
    loads = []
    for blk in fn.blocks:
        keep = []
        for ins in blk.instructions:
            if type(ins).__name__ == "InstLoadActFuncSet":
                loads.append(ins)
            else:
                keep.append(ins)
        blk.instructions = keep
    if loads:
        main = fn.blocks[1]
        main.instructions = loads + main.instructions
    # The trigger can carry only one codegen sync wait; point it at the ACT
    # engine sem tick of the last staging copy (the prep's descriptor gen on
    # the Pool engine finishes several microseconds earlier, so dropping the
    # Pool tick wait is safe).  Delete the placeholder gate entirely.
    act_total = 0
    last_src_tick = None
    trig = None
    gate = None
    for blk in fn.blocks:
        for ins in blk.instructions:
            if type(ins).__name__ == "InstTriggerDma":
                trig = ins
            si = ins.sync_info
            if si is None:
                continue
            for w in si.on_wait:
                if w.ant_name == "actdone":
                    gate = ins
            for u in si.on_update:
                if (u.ant_name or "").startswith("DVE_"):
                    act_total += (u.update_value or 1)
                    if type(ins).__name__ == "InstTensorCopy":
                        last_src_tick = (u.id, u.ant_name, act_total)
    assert trig is not None and last_src_tick is not None, (trig, last_src_tick)
    sid, sname, val = last_src_tick
    trig.sync_info.on_wait = [mybir.SyncWait(
        sync_type="semaphore", id=sid, ant_name=sname,
        wait_mode="sem-ge-imm", wait_value=val, wait_reg=None)]
    if gate is not None:
        for blk in fn.blocks:
            blk.instructions = [i for i in blk.instructions if i is not gate]
    # Hoist the blob HWDGE DMA (SP) and the first stream DMA (Pool) ahead of
    # the entry barrier: their descriptors have no dependencies, so the first
    # transfer starts ~1.3us in instead of ~2.2us.
    main = fn.blocks[1]
    hoist = []
    n_sp = n_pool = 0
    keep = []
    for ins in main.instructions:
        if (type(ins).__name__ == "InstDMACopy" and n_sp < 2
                and ins.engine == mybir.EngineType.SP):
            hoist.append(ins)
            n_sp += 1
        elif (type(ins).__name__ == "InstDMACopy" and n_pool < 1
                and ins.engine == mybir.EngineType.Pool):
            hoist.append(ins)
            n_pool += 1
        else:
            keep.append(ins)
    main.instructions = keep
    fn.blocks[0].instructions = hoist + fn.blocks[0].instructions
    # Exit-barrier trim: the writeback completion is already enforced by the
    # per-engine DMASW waits in the exit block; the trailing gather/release
    # barrier rounds only synchronize engine end times.  Drop them so the
    # kernel ends when the last DMASW waiter releases.
    exit_blk = fn.blocks[-1]
    exit_blk.instructions = [
        i for i in exit_blk.instructions
        if not i.name.startswith("barrier_")
    ]
    return nc


_NC_CACHE = None


def _get_program():
    global _NC_CACHE
    if _NC_CACHE is None:
        _NC_CACHE = _build_program()
    return _NC_CACHE


_RUNNER_CACHE = None


def _get_runner():
    """Compiled-once jit(shard_map) executor over 8 cores."""
    global _RUNNER_CACHE
    if _RUNNER_CACHE is None:
        import jax
        from jax.sharding import Mesh, PartitionSpec
        from jax.experimental.shard_map import shard_map
        from concourse import bass2jax, mybir as mb

        nc = _get_program()
        bass2jax.install_neuronx_cc_hook()
        partition_name = (nc.partition_id_tensor.name
                          if nc.partition_id_tensor else None)
        in_names, out_names, out_avals = [], [], []
        for alloc in nc.m.functions[0].allocations:
            if not isinstance(alloc, mb.MemoryLocationSet):
                continue
            name = alloc.memorylocations[0].name
            if alloc.kind == "ExternalInput":
                if name != partition_name:
                    in_names.append(name)
            elif alloc.kind == "ExternalOutput":
                out_names.append(name)
                out_avals.append(jax.core.ShapedArray(
                    tuple(alloc.tensor_shape), mb.dt.np(alloc.dtype)))
        n_params = len(in_names)
        all_names = in_names + out_names
        if partition_name is not None:
            all_names = all_names + [partition_name]

        def _body(*args):
            operands = list(args)
            if partition_name is not None:
                operands.append(bass2jax.partition_id_tensor())
            outs = bass2jax._bass_exec_p.bind(
                *operands,
                out_avals=tuple(out_avals),
                in_names=tuple(all_names),
                out_names=tuple(out_names),
                lowering_input_output_aliases=(),
                sim_require_finite=True,
                sim_require_nnan=True,
                nc=nc,
            )
            return tuple(outs)

        devices = jax.devices()[:8]
        mesh = Mesh(np.asarray(devices), ("core",))
        n_outs = len(out_names)
        sharded = jax.jit(
            shard_map(_body, mesh=mesh,
                      in_specs=(PartitionSpec("core"),) * (n_params + n_outs),
                      out_specs=(PartitionSpec("core"),) * n_outs,
                      check_rep=False),
            donate_argnums=tuple(range(n_params, n_params + n_outs)),
            keep_unused=True,
        )
        _RUNNER_CACHE = (sharded, in_names[:n_params], out_names, out_avals)
    return _RUNNER_CACHE


def _host_side(consumption, generation, sharing_matrix, lv_group_ids,
               valid_lv_mask):
    """Shared input prep: per-core input maps."""
    consumption = np.ascontiguousarray(consumption, dtype=np.float32)
    generation = np.ascontiguousarray(generation, dtype=np.float32)
    sharing_matrix = np.ascontiguousarray(sharing_matrix, dtype=np.float32)
    ids = np.asarray(lv_group_ids)
    valid = np.asarray(valid_lv_mask, dtype=np.float32)

    onehot = (ids[None, :] == np.arange(G)[:, None]).astype(np.float32)
    n_unique = np.float32(np.unique(ids).size)
    M = onehot * valid[None, :]                      # [G, N]
    mt = np.ascontiguousarray(M.T)                   # [N, G]

    # cns[msub, j, k, g] = -M[g, msub*Q + 2j + k]  (negated imports), plus
    # the [16, 128] replicate indicator ind[p, pp] = (pp % 16 == p).
    cw = (-mt).reshape(C, Q * G)                     # [msub, (q, g)]
    cn16 = cw.astype(ml_dtypes.float8_e4m3).view(np.uint8)
    ind = (np.arange(P)[None, :] % C == np.arange(C)[:, None]).astype(
        np.float32).astype(ml_dtypes.float8_e4m3).view(np.uint8)
    cns = np.ascontiguousarray(
        np.concatenate([cn16, ind], axis=1))         # [16, CNSW]

    in_maps = []
    for c in range(8):
        b, hh = divmod(c, 2)
        sl = slice(hh * NLOC, (hh + 1) * NLOC)
        mt_half = mt[sl]                             # [NLOC, G]
        # rowL[p=(na,msub), blk, g] = M[g, blk*A + na]
        proj = mt_half.reshape(NBLK, A, G).transpose(1, 0, 2)   # [na, blk, g]
        proj = np.broadcast_to(proj[:, None], (A, C, NBLK, G))
        rowl8 = proj.reshape(P, NBLK * G).astype(
            ml_dtypes.float8_e4m3).view(np.uint8)
        # smt[p, nb, f]: f = [mt row | cons row | gen row] for n = nb*P + p
        sm = np.empty((2, P, SMW), np.float16)
        sm[:, :, :G] = mt_half.reshape(2, P, G)
        sm[:, :, G:G + T] = consumption[b, sl].reshape(2, P, T)
        sm[:, :, G + T:] = generation[b, sl].reshape(2, P, T)
        sm_bytes = sm.transpose(1, 0, 2).reshape(P, -1).view(np.uint8)

        blob = np.zeros((P, BLOBW), np.uint8)
        blob[:, :BLOB_ROWL] = rowl8
        blob[:, BLOB_ROWL:BLOB_ROWL + BLOB_SMT] = sm_bytes
        in_maps.append({
            "s": np.ascontiguousarray(sharing_matrix[b, sl]),
            "blob": np.ascontiguousarray(blob),
            "cns": cns,
        })
    return in_maps, n_unique


def kernel(consumption, generation, sharing_matrix, lv_group_ids,
           valid_lv_mask, imbalance_penalty_weight, _want_results=False,
           **run_kwargs):
    w = np.float32(np.asarray(imbalance_penalty_weight))
    in_maps, n_unique = _host_side(consumption, generation, sharing_matrix,
                                   lv_group_ids, valid_lv_mask)
    res = None
    if _want_results or run_kwargs:
        nc = _get_program()
        res = run_bass_kernel_spmd(nc, in_maps, core_ids=list(range(8)),
                                   **run_kwargs)
        parts = np.stack([res.results[c]["out"] for c in range(8)])
    else:
        try:
            fn, in_names, out_names, out_avals = _get_runner()
            concat_in = [np.concatenate([m[name] for m in in_maps], axis=0)
                         for name in in_names]
            zeros = [np.zeros((8 * a.shape[0], *a.shape[1:]), a.dtype)
                     for a in out_avals]
            out_arrs = fn(*concat_in, *zeros)
            parts = np.asarray(out_arrs[out_names.index("out")]).reshape(
                8, P, OW)
        except Exception:
            nc = _get_program()
            res = run_bass_kernel_spmd(nc, in_maps, core_ids=list(range(8)))
            parts = np.stack([res.results[c]["out"] for c in range(8)]).reshape(
                8, P, OW)
    # partition p (< G) carries [gc_p | gg_p | net_p] as 3*T columns
    per_core = parts[:, :G, :].reshape(8, G, 3, T).transpose(0, 2, 1, 3)
    full = per_core.reshape(B, 2, 3, G, T).sum(axis=1, dtype=np.float32)
    gc, gg, net = full[:, 0], full[:, 1], full[:, 2]

    imbalance = np.abs(gc - gg + net)
    total = gc + gg + np.float32(EPS)
    pen = np.maximum(imbalance / total - np.float32(TOL), np.float32(0))
    outv = np.float32(pen.sum(dtype=np.float32) * w / n_unique)
    out_arr = np.array(outv, dtype=np.float32)
    if _want_results:
        return out_arr, res
    return out_arr
